# revision 1
# baseline (speedup 1.0000x reference)
"""Trainium2 Bass kernel for nn_CrossEncoderLongformer (6-layer Longformer
cross-encoder, L=4096, H=768, 12 heads, W=256 sliding window, 64 global
tokens, B=1).

Sequence-sharded SPMD over 8 NeuronCores (512 tokens/core), feature-major
activations, per-layer halo AllGathers + flash-combined global rows.
Self-contained: host does embedding gather, LN_emb and the ranking head.
"""
import contextlib
import math
import os
import sys

if '/opt/trn_rl_repo' not in sys.path:
    sys.path.insert(0, '/opt/trn_rl_repo')

import numpy as np

import concourse.bass as bass
import concourse.bacc as bacc
import concourse.tile as tile
from concourse import mybir
from concourse.bass_utils import run_bass_kernel_spmd

H, NH, NL, FF, W, CMAX, VOC, L, G = 768, 12, 6, 3072, 256, 32, 50272, 4096, 64
DH = H // NH
SCALE = 1.0 / math.sqrt(DH)
NCORE = 8
T = L // NCORE                # 512
NBLK = T // W                 # 2
FT = H // 128                 # 6
TEXT = T + 2 * W              # 1024
SK = 3 * W + G                # 832 score columns
NEG = -1e9
OOB = 1 << 28

f32 = mybir.dt.float32
f32r = mybir.dt.float32r
bf16 = mybir.dt.bfloat16
i32 = mybir.dt.int32
Alu = mybir.AluOpType
Act = mybir.ActivationFunctionType
AX = mybir.AxisListType

N_LAYERS = int(os.environ.get("KERNEL_LAYERS", str(NL)))

AGA = [[0, 1], [2, 3], [4, 5], [6, 7]]
AGB = [[0, 7], [1, 2], [3, 4], [5, 6]]
AG8 = [list(range(NCORE))]


# ---------------------------------------------------------------- host side

def _np_ln(x, s, b, eps=1e-5):
    m = x.mean(-1, keepdims=True)
    v = ((x - m) ** 2).mean(-1, keepdims=True)
    return (x - m) / np.sqrt(v + eps) * s + b


def _np_gelu(x):
    try:
        from scipy.special import erf
        return 0.5 * x * (1.0 + erf(x / math.sqrt(2.0)))
    except Exception:
        e = np.vectorize(math.erf)
        return 0.5 * x * (1.0 + e(x / math.sqrt(2.0)))


def _featpack(x):
    """[N, H] -> [FT, 128, N]."""
    return np.ascontiguousarray(x.T.reshape(FT, 128, -1))


def host_prep(inputs):
    ids = np.asarray(inputs['input_ids'])
    am = np.asarray(inputs['attention_mask'])[0].astype(bool)
    gpos = np.asarray(inputs['global_positions']).astype(np.int64)
    emb = (np.asarray(inputs['emb_tok'])[ids[0]]
           + np.asarray(inputs['emb_pos'])[:L]).astype(np.float32)
    x0 = _np_ln(emb, np.asarray(inputs['ln_emb_s']), np.asarray(inputs['ln_emb_b']))

    is_glob = np.zeros(L, bool)
    is_glob[gpos] = True

    rel = np.arange(3 * W)[None, :] - W - np.arange(W)[:, None]
    band = np.abs(rel) <= W

    last_slot = {}
    for g, p in enumerate(gpos):
        last_slot[int(p)] = g

    x0p = np.pad(x0, ((W, W), (0, 0)))
    per_core = []
    for c in range(NCORE):
        d = {}
        d['x0_ext'] = _featpack(x0p[c * T: c * T + TEXT]).astype(np.float32)

        mask_add = np.zeros((NBLK, 2, 128, SK), np.float32)
        for b in range(NBLK):
            gb = c * NBLK + b
            pos = gb * W + np.arange(3 * W) - W
            inb = (pos >= 0) & (pos < L)
            safe = np.clip(pos, 0, L - 1)
            key_ok = inb & (~is_glob[safe]) & am[safe]
            mloc = np.where(key_ok[None, :] & band, 0.0, NEG / SCALE)
            for qt in range(2):
                mask_add[b, qt, :, :3 * W] = mloc[qt * 128:(qt + 1) * 128]
        import ml_dtypes
        d['maskadd'] = mask_add.astype(ml_dtypes.bfloat16)

        amrow = np.where(am[c * T:(c + 1) * T], 0.0, NEG / SCALE).astype(np.float32)
        d['amadd'] = np.broadcast_to(amrow, (64, T)).copy()

        S = np.zeros((T, G), np.float32)
        for g, p in enumerate(gpos):
            p = int(p)
            if p // T == c:
                S[p % T, g] = 1.0
        d['ssel'] = np.ascontiguousarray(S.reshape(4, 128, G))

        SgT = np.zeros((G, T), np.float32)
        for p, g in last_slot.items():
            if p // T == c:
                SgT[g, p % T] = 1.0
        d['sgt'] = SgT

        zm = np.ones(T, np.float32)
        for p in gpos:
            p = int(p)
            if p // T == c:
                zm[p % T] = 0.0
        d['zmaskc'] = np.ascontiguousarray(zm.reshape(4, 128).T)

        # halo receive offsets into gath8 [8*1536, W]:
        # row = nbr*1536 + side*768 + p*6 + ft
        offs = np.zeros((128, 2, FT), np.int32)
        p_ar = np.arange(128)
        for combo in range(2):
            if combo == 0:              # left halo <- left neighbor's right edge
                if c == 0:
                    continue            # junk row 0, keys masked by inb
                nbr, side = c - 1, 1
            else:                       # right halo <- right neighbor's left edge
                if c == NCORE - 1:
                    continue
                nbr, side = c + 1, 0
            for ft in range(FT):
                offs[:, combo, ft] = nbr * 1536 + side * 768 + p_ar * 6 + ft
        d['offs'] = offs
        d['xg0_tok'] = x0[gpos].astype(np.float32)
        per_core.append(d)

    return per_core, dict(ids=ids, am=am, gpos=gpos)


# ------------------------------------------------------------- the program

class Env:
    pass


def build_program(n_layers=N_LAYERS):
    nc = bacc.Bacc("TRN2", target_bir_lowering=False, debug=False,
                   enable_asserts=True, num_devices=NCORE)
    e = Env()
    e.nc = nc
    e.n_layers = n_layers

    def din(name, shape, dt=f32r):
        return nc.dram_tensor(name, list(shape), dt, kind="ExternalInput").ap()

    for n in ('wq', 'wk', 'wv', 'wqg', 'wkg', 'wvg', 'wo'):
        setattr(e, n, din(n, [NL, H, H]))
    e.w1 = din('w1', [NL, H, FF])
    e.w2 = din('w2', [NL, FF, H])
    for n in ('bq', 'bk', 'bv', 'bqg', 'bkg', 'bvg', 'bo', 'b2',
              'ln1s', 'ln1b', 'ln2s', 'ln2b'):
        setattr(e, n, din(n, [NL, H], f32))
    e.b1 = din('b1', [NL, FF], f32)

    e.x0_ext = din('x0_ext', [FT, 128, TEXT])
    e.xg0_tok = din('xg0_tok', [G, H])
    e.maskadd = din('maskadd', [NBLK, 2, 128, SK], bf16)
    e.amadd = din('amadd', [64, T], f32)
    e.ssel_i = din('ssel', [4, 128, G])
    e.sgt_i = din('sgt', [G, T])
    e.zmaskc = din('zmaskc', [128, 4], f32)
    e.offs_i = din('offs', [128, 2, FT], i32)
    e.cfr_i = din('cfr', [128, 129], f32r)
    e.cbf_i = din('cbf', [128, 128], bf16)
    e.eps_i = din('eps', [1, 1], f32)

    e.xout = nc.dram_tensor('xout', [FT, 128, T], f32r, kind="ExternalOutput").ap()

    with tile.TileContext(nc) as tc:
        e.tc = tc
        with contextlib.ExitStack() as stack:
            pers = stack.enter_context(tc.tile_pool(name="pers", bufs=1))
            dram = stack.enter_context(tc.tile_pool(name="dram", bufs=1, space="DRAM"))
            e.pers = pers
            e.dram = dram

            e.x_ext = pers.tile([128, FT, TEXT], f32r, tag="x_ext")
            e.mask_sb = pers.tile([128, NBLK, 2, SK], bf16, tag="mask")
            e.am_sb = pers.tile([64, T], f32, tag="am")
            e.ssel_sb = pers.tile([128, 4, G], f32r, tag="ssel")
            e.sgt_sb = pers.tile([64, T], f32r, tag="sgt")
            e.zmc_sb = pers.tile([128, 4], f32, tag="zmc")
            e.offs_sb = pers.tile([128, 2, FT], i32, tag="offs")
            e.cfr = pers.tile([128, 129], f32r, tag="cfr")
            e.id_bf = pers.tile([128, 128], bf16, tag="idbf")
            e.eps_sb = pers.tile([1, 1], f32, tag="eps")
            e.xg_tok = pers.tile([G, H], f32r, tag="xg_tok")
            e.id_fr = e.cfr[:, 0:128]
            e.ones_sb = e.cfr[:, 128:129]

            nc.sync.dma_start(out=e.cfr[:, :], in_=e.cfr_i[:, :])
            nc.sync.dma_start(out=e.id_bf[:, :], in_=e.cbf_i[:, :])
            nc.sync.dma_start(out=e.eps_sb[:, :], in_=e.eps_i[:, :])

            for ft in range(FT):
                nc.sync.dma_start(out=e.x_ext[:, ft, :], in_=e.x0_ext[ft])
            for b in range(NBLK):
                for qt in range(2):
                    nc.sync.dma_start(out=e.mask_sb[:, b, qt, :], in_=e.maskadd[b, qt])
            nc.sync.dma_start(out=e.am_sb[:, :], in_=e.amadd[:, :])
            for kt in range(4):
                nc.sync.dma_start(out=e.ssel_sb[:, kt, :], in_=e.ssel_i[kt])
            nc.sync.dma_start(out=e.sgt_sb[:, :], in_=e.sgt_i[:, :])
            nc.sync.dma_start(out=e.zmc_sb[:, :], in_=e.zmaskc[:, :])
            nc.sync.dma_start(out=e.offs_sb[:, :, :], in_=e.offs_i[:, :, :])
            nc.sync.dma_start(out=e.xg_tok[:, :], in_=e.xg0_tok[:, :])

            e.edges_d = dram.tile([2, 128, FT, W], bf16, tag="edges")
            pass  # gath8 allocated per layer
            e.xgc_d = dram.tile([G, H], f32r, tag="xgc")
            pass  # xgg allocated per layer
            e.pc_d = dram.tile([G, H + 24], f32, tag="pc")
            pass  # pg allocated per layer
            e.stats_d = dram.tile([2, T], f32, tag="statsd")

            for li in range(n_layers):
                with nc.named_scope(f"layer{li}"):
                    _layer(e, li)

            for ft in range(FT):
                nc.sync.dma_start(out=e.xout[ft], in_=e.x_ext[:, ft, W:W + T])

    nc.compile()
    return nc


def _bcast(ap, n):
    """Broadcast an AP along a new leading (partition) axis of size n."""
    return bass.AP(tensor=ap.tensor, offset=ap.offset, ap=[[0, n]] + list(ap.ap))


def _layer(e, li):
    nc, tc = e.nc, e.tc
    OWN = slice(W, W + T)

    with contextlib.ExitStack() as ctx:
        lay = ctx.enter_context(tc.tile_pool(name=f"lay{li}", bufs=1))
        wpool = ctx.enter_context(tc.tile_pool(name=f"w{li}", bufs=1))
        wff = ctx.enter_context(tc.tile_pool(name=f"wff{li}", bufs=2))
        sc = ctx.enter_context(tc.tile_pool(name=f"sc{li}", bufs=2))
        scl = ctx.enter_context(tc.tile_pool(name=f"scl{li}", bufs=1))
        pp = ctx.enter_context(tc.tile_pool(name=f"pp{li}", bufs=2))

        # ---- per-layer bias / ln param tiles
        def bias_tile(src, cols, tag):
            t = lay.tile([128, cols], f32, tag=tag)
            nc.sync.dma_start(out=t[:, :],
                             in_=src[li].rearrange("(f p) -> p f", p=128))
            return t

        bq_sb = bias_tile(e.bq, FT, "bq")
        bk_sb = bias_tile(e.bk, FT, "bk")
        bqg_sb = bias_tile(e.bqg, FT, "bqg")
        bkg_sb = bias_tile(e.bkg, FT, "bkg")
        bo_sb = bias_tile(e.bo, FT, "bo")
        b2_sb = bias_tile(e.b2, FT, "b2")
        b1_sb = bias_tile(e.b1, FF // 128, "b1")
        ln1s_sb = bias_tile(e.ln1s, FT, "ln1s")
        ln1b_sb = bias_tile(e.ln1b, FT, "ln1b")
        ln2s_sb = bias_tile(e.ln2s, FT, "ln2s")
        ln2b_sb = bias_tile(e.ln2b, FT, "ln2b")

        bv_exp = lay.tile([128, H], f32, tag="bvexp")
        nc.sync.dma_start(out=bv_exp[:, :], in_=_bcast(e.bv[li], 128))
        bvg_exp = lay.tile([128, H], f32, tag="bvgexp")
        nc.sync.dma_start(out=bvg_exp[:, :], in_=_bcast(e.bvg[li], 128))

        # ==================== P1: receive, global projections, partials, qkv
        with tc.tile_pool(name=f"P1_{li}", bufs=1, space="PSUM") as P1:
            def p1(shape, tag, bufs_tag=None):
                return P1.tile(shape, f32, tag=tag)

            if li > 0:
                for combo, sl in enumerate([slice(0, W), slice(W + T, TEXT)]):
                    for ft in range(FT):
                        nc.gpsimd.indirect_dma_start(
                            out=e.x_ext[:, ft, sl],
                            out_offset=None,
                            in_=e.gath8_d[:, :],
                            in_offset=bass.IndirectOffsetOnAxis(
                                ap=e.offs_sb[:, combo, ft:ft + 1], axis=0),
                        )
                for c in range(NCORE):
                    xgch = pp.tile([G, H], f32r, tag="gh64")
                    nc.sync.dma_start(out=xgch[:, :],
                                     in_=e.xgg_d[c * G:(c + 1) * G, :])
                    if c == 0:
                        nc.vector.tensor_copy(out=e.xg_tok[:, :], in_=xgch[:, :])
                    else:
                        nc.vector.tensor_add(out=e.xg_tok[:, :],
                                             in0=e.xg_tok[:, :], in1=xgch[:, :])

            xg_feat = lay.tile([128, FT, G], f32r, tag="xg_feat")
            for ft in range(FT):
                tp = P1.tile([128, G], f32r, tag="tp")
                nc.tensor.transpose(out=tp[:, :],
                                    in_=e.xg_tok[:, ft * 128:(ft + 1) * 128],
                                    identity=e.id_fr[0:64, 0:64])
                nc.scalar.copy(out=xg_feat[:, ft, :], in_=tp[:, :])

            def load_w(src):
                t = wpool.tile([128, FT, H], f32r, tag="wres")
                for kt in range(FT):
                    nc.sync.dma_start(out=t[:, kt, :],
                                     in_=src[li, kt * 128:(kt + 1) * 128, :])
                return t

            def proj_small(wres, bias, out):
                for ot in range(FT):
                    acc = P1.tile([128, G], f32, tag="acc")
                    for kt in range(FT):
                        nc.tensor.matmul(acc[:, :],
                                         wres[:, kt, ot * 128:(ot + 1) * 128],
                                         xg_feat[:, kt, :], start=(kt == 0),
                                         stop=(kt == FT - 1))
                    nc.vector.tensor_scalar_add(out=out[:, ot, :], in0=acc[:, :],
                                                scalar1=bias[:, ot:ot + 1])

            def proj_feat(wres, bias, out, src_cols, dst_cols, n):
                for ot in range(FT):
                    acc = P1.tile([128, 512], f32, tag="acc")
                    for kt in range(FT):
                        nc.tensor.matmul(acc[:, :n],
                                         wres[:, kt, ot * 128:(ot + 1) * 128],
                                         e.x_ext[:, kt, src_cols],
                                         start=(kt == 0), stop=(kt == FT - 1))
                    nc.vector.tensor_scalar_add(out=out[:, ot, dst_cols],
                                                in0=acc[:, :n],
                                                scalar1=bias[:, ot:ot + 1])

            def proj_tok(wres, bias_exp, out, tchunks, col0):
                for tc_ in tchunks:
                    for half in range(2):
                        hs = slice(half * 384, (half + 1) * 384)
                        acc = P1.tile([128, 384], f32, tag="acc")
                        cs = slice(col0 + tc_ * 128, col0 + (tc_ + 1) * 128)
                        for kt in range(FT):
                            nc.tensor.matmul(acc[:, :], e.x_ext[:, kt, cs],
                                             wres[:, kt, hs], start=(kt == 0),
                                             stop=(kt == FT - 1))
                        nc.vector.tensor_add(out=out[:, tc_, hs], in0=acc[:, :],
                                             in1=bias_exp[:, hs])

            # Wkg
            wres = load_w(e.wkg)
            kgs_feat = lay.tile([128, FT, G], bf16, tag="kgs")
            proj_small(wres, bkg_sb, kgs_feat)
            kg_own = lay.tile([128, FT, T], bf16, tag="kg_own")
            proj_feat(wres, bkg_sb, kg_own, OWN, slice(0, T), T)

            # Wvg
            wres = load_w(e.wvg)
            vgs_tok = lay.tile([64, H], bf16, tag="vgs")
            for half in range(2):
                hs = slice(half * 384, (half + 1) * 384)
                acc = P1.tile([64, 384], f32, tag="accW")
                for kt in range(FT):
                    nc.tensor.matmul(acc[:, :], xg_feat[:, kt, :], wres[:, kt, hs],
                                     start=(kt == 0), stop=(kt == FT - 1))
                nc.vector.tensor_add(out=vgs_tok[:, hs], in0=acc[:, :],
                                     in1=bvg_exp[0:64, hs])
            vg_own = lay.tile([128, 4, H], bf16, tag="vg_own")
            proj_tok(wres, bvg_exp, vg_own, range(4), W)

            # Wqg
            wres = load_w(e.wqg)
            qg_feat = lay.tile([128, FT, G], bf16, tag="qg")
            proj_small(wres, bqg_sb, qg_feat)

            # ---- flash partials + AllGather
            contrib = lay.tile([64, H + 24], f32, tag="contrib")
            for h in range(NH):
                hp, ht = (h % 2) * 64, h // 2
                sp = P1.tile([64, T], f32, tag="accW")
                nc.tensor.matmul(sp[:, :], qg_feat[hp:hp + 64, ht, :],
                                 kg_own[hp:hp + 64, ht, :], start=True, stop=True)
                s_sb = sc.tile([64, T], f32, tag="s_sb")
                nc.vector.tensor_add(out=s_sb[:, :], in0=sp[:, :], in1=e.am_sb[:, :])
                mx = sc.tile([64, 1], f32, tag="mx")
                nc.vector.reduce_max(out=mx[:, :], in_=s_sb[:, :], axis=AX.X)
                mneg = sc.tile([64, 1], f32, tag="mneg")
                nc.vector.tensor_scalar_mul(out=mneg[:, :], in0=mx[:, :],
                                            scalar1=-SCALE)
                pb = sc.tile([64, T], bf16, tag="p")
                lsum = sc.tile([64, 1], f32, tag="lsum")
                nc.scalar.activation(out=pb[:, :], in_=s_sb[:, :], func=Act.Exp,
                                     bias=mneg[:, :], scale=SCALE,
                                     accum_out=lsum[:, :])
                nc.vector.tensor_scalar_mul(out=contrib[:, H + h:H + h + 1],
                                            in0=mx[:, :], scalar1=SCALE)
                nc.vector.tensor_copy(out=contrib[:, H + 12 + h:H + 12 + h + 1],
                                      in_=lsum[:, :])
                pT = pp.tile([128, 4, 64], bf16, tag="pT")
                for j in range(4):
                    tp = P1.tile([128, 64], bf16, tag="tpb")
                    nc.tensor.transpose(out=tp[:, :],
                                        in_=pb[:, j * 128:(j + 1) * 128],
                                        identity=e.id_bf[:64, :64])
                    nc.scalar.copy(out=pT[:, j, :], in_=tp[:, :])
                oc = P1.tile([64, 64], f32, tag="accW")
                for j in range(4):
                    nc.tensor.matmul(oc[:, :], pT[:, j, :],
                                     vg_own[:, j, h * 64:(h + 1) * 64],
                                     start=(j == 0), stop=(j == 3))
                nc.scalar.copy(out=contrib[:, h * 64:(h + 1) * 64], in_=oc[:, :])
            nc.sync.dma_start(out=e.pc_d[:, :], in_=contrib[:, :])
            e.pg_d = e.dram.tile([NCORE * G, H + 24], f32, tag=f"pg{li}",
                                 addr_space="Shared")
            nc.gpsimd.collective_compute(
                "AllGather", Alu.bypass, replica_groups=AG8,
                ins=[e.pc_d.opt()], outs=[e.pg_d.opt()])

            # ---- own q/k/v
            wres = load_w(e.wq)
            q_sb = lay.tile([128, FT, T], bf16, tag="q_z")
            proj_feat(wres, bq_sb, q_sb, OWN, slice(0, T), T)

            wres = load_w(e.wk)
            k_ext = lay.tile([128, FT, TEXT], bf16, tag="k_ext")
            proj_feat(wres, bk_sb, k_ext, OWN, slice(W, W + T), T)
            proj_feat(wres, bk_sb, k_ext, slice(0, W), slice(0, W), W)
            proj_feat(wres, bk_sb, k_ext, slice(W + T, TEXT), slice(W + T, TEXT), W)

            wres = load_w(e.wv)
            v_tok = lay.tile([128, 8, H], bf16, tag="v_tok")
            proj_tok(wres, bv_exp, v_tok, [2, 3, 4, 5, 0, 1, 6, 7], 0)

        # ==================== P2: local attention
        out_attn = lay.tile([128, FT, T], f32r, tag="scr12")
        with tc.tile_pool(name=f"P2_{li}", bufs=1, space="PSUM") as P2:
            for b in range(NBLK):
                for h in range(NH):
                    hp, ht = (h % 2) * 64, h // 2
                    pboth = []
                    for qt in range(2):
                        qs = slice(b * W + qt * 128, b * W + (qt + 1) * 128)
                        spsum = P2.tile([128, SK], f32, tag=f"score{qt}")
                        nc.tensor.matmul(spsum[:, 0:512],
                                         q_sb[hp:hp + 64, ht, qs],
                                         k_ext[hp:hp + 64, ht, b * W:b * W + 512],
                                         start=True, stop=True)
                        nc.tensor.matmul(spsum[:, 512:3 * W],
                                         q_sb[hp:hp + 64, ht, qs],
                                         k_ext[hp:hp + 64, ht,
                                               b * W + 512:b * W + 3 * W],
                                         start=True, stop=True)
                        nc.tensor.matmul(spsum[:, 3 * W:SK],
                                         q_sb[hp:hp + 64, ht, qs],
                                         kgs_feat[hp:hp + 64, ht, :],
                                         start=True, stop=True)
                        s_sb = sc.tile([128, SK], f32, tag="s_sb")
                        nc.vector.tensor_add(out=s_sb[:, :], in0=spsum[:, :],
                                             in1=e.mask_sb[:, b, qt, :])
                        mx = sc.tile([128, 1], f32, tag="mx")
                        nc.vector.reduce_max(out=mx[:, :], in_=s_sb[:, :], axis=AX.X)
                        mneg = sc.tile([128, 1], f32, tag="mneg")
                        nc.vector.tensor_scalar_mul(out=mneg[:, :], in0=mx[:, :],
                                                    scalar1=-SCALE)
                        pb = sc.tile([128, SK], bf16, tag="p")
                        den = sc.tile([128, 1], f32, tag="lsum")
                        nc.scalar.activation(out=pb[:, :], in_=s_sb[:, :],
                                             func=Act.Exp, bias=mneg[:, :],
                                             scale=SCALE, accum_out=den[:, :])
                        rec = sc.tile([128, 1], f32, tag="rec")
                        nc.vector.reciprocal(out=rec[:, :], in_=den[:, :])
                        rz = sc.tile([128, 1], f32, tag="rz")
                        nc.vector.tensor_tensor(
                            out=rz[:, :], in0=rec[:, :],
                            in1=e.zmc_sb[:, b * 2 + qt:b * 2 + qt + 1], op=Alu.mult)
                        nc.vector.tensor_scalar_mul(out=pb[:, :], in0=pb[:, :],
                                                    scalar1=rz[:, :])
                        pboth.append(pb)
                    ov = P2.tile([64, 2 * 128], f32, tag="ov")
                    for j in range(6):
                        aT = P2.tile([128, 2 * 128], bf16, tag="aT")
                        aTsb = pp.tile([128, 2 * 128], bf16, tag="aT_sb")
                        for qt in range(2):
                            nc.tensor.transpose(
                                out=aT[:, qt * 128:(qt + 1) * 128],
                                in_=pboth[qt][:, j * 128:(j + 1) * 128],
                                identity=e.id_bf[:, :])
                        nc.scalar.copy(out=aTsb[:, :], in_=aT[:, :])
                        nc.tensor.matmul(ov[:, :],
                                         v_tok[:, 2 * b + j, h * 64:(h + 1) * 64],
                                         aTsb[:, :], start=(j == 0), stop=False,
                                         skip_group_check=True)
                    aTg = P2.tile([64, 2 * 128], bf16, tag="aTg")
                    aTgsb = pp.tile([64, 2 * 128], bf16, tag="aTg_sb")
                    for qt in range(2):
                        nc.tensor.transpose(out=aTg[:, qt * 128:(qt + 1) * 128],
                                            in_=pboth[qt][:, 3 * W:SK],
                                            identity=e.id_bf[:, :])
                    nc.scalar.copy(out=aTgsb[:, :], in_=aTg[:, :])
                    nc.tensor.matmul(ov[:, :], vgs_tok[:, h * 64:(h + 1) * 64],
                                     aTgsb[:, :], start=False, stop=True,
                                     skip_group_check=True)
                    nc.scalar.copy(out=out_attn[hp:hp + 64, ht, b * W:(b + 1) * W],
                                   in_=ov[:, :])

        # ==================== P3: combine partials, og, Wo, LN1
        z_sb = lay.tile([128, FT, T], f32r, tag="q_z")
        x_mid = lay.tile([128, FT, T], f32r, tag="x_mid")
        with tc.tile_pool(name=f"P3_{li}", bufs=1, space="PSUM") as P3:
            stat_all = lay.tile([64, NCORE, 24], f32, tag="stat_all")
            for c in range(NCORE):
                nc.sync.dma_start(out=stat_all[:, c, :],
                                 in_=e.pg_d[c * G:(c + 1) * G, H:H + 24])
            mg = scl.tile([64, 12], f32, tag="mg")
            nc.vector.tensor_copy(out=mg[:, :], in_=stat_all[:, 0, 0:12])
            for c in range(1, NCORE):
                nc.vector.tensor_tensor(out=mg[:, :], in0=mg[:, :],
                                        in1=stat_all[:, c, 0:12], op=Alu.max)
            wts = lay.tile([64, NCORE, 12], f32, tag="wts")
            lsum_g = scl.tile([64, 12], f32, tag="lsum_g")
            for c in range(NCORE):
                al = scl.tile([64, 12], f32, tag="alpha")
                nc.vector.tensor_tensor(out=al[:, :], in0=stat_all[:, c, 0:12],
                                        in1=mg[:, :], op=Alu.subtract)
                nc.scalar.activation(out=wts[:, c, :], in_=al[:, :], func=Act.Exp,
                                     bias=0.0, scale=1.0)
                lw = scl.tile([64, 12], f32, tag="lw")
                nc.vector.tensor_tensor(out=lw[:, :], in0=wts[:, c, :],
                                        in1=stat_all[:, c, 12:24], op=Alu.mult)
                if c == 0:
                    nc.vector.tensor_copy(out=lsum_g[:, :], in_=lw[:, :])
                else:
                    nc.vector.tensor_add(out=lsum_g[:, :], in0=lsum_g[:, :],
                                         in1=lw[:, :])
            rl = scl.tile([64, 12], f32, tag="rl")
            nc.vector.reciprocal(out=rl[:, :], in_=lsum_g[:, :])
            og_tok = lay.tile([64, H], f32r, tag="og_tok")
            for c in range(NCORE):
                wc = scl.tile([64, 12], f32, tag="wc")
                nc.vector.tensor_tensor(out=wc[:, :], in0=wts[:, c, :],
                                        in1=rl[:, :], op=Alu.mult)
                och = pp.tile([64, H], f32, tag="gh64")
                nc.sync.dma_start(out=och[:, :], in_=e.pg_d[c * G:(c + 1) * G, 0:H])
                for hh in range(NH):
                    nc.vector.tensor_scalar_mul(
                        out=och[:, hh * 64:(hh + 1) * 64],
                        in0=och[:, hh * 64:(hh + 1) * 64],
                        scalar1=wc[:, hh:hh + 1])
                if c == 0:
                    nc.vector.tensor_copy(out=og_tok[:, :], in_=och[:, :])
                else:
                    nc.vector.tensor_add(out=og_tok[:, :], in0=og_tok[:, :],
                                         in1=och[:, :])
            og_feat = lay.tile([128, FT, G], f32r, tag="og_feat")
            for ft in range(FT):
                tp = P3.tile([128, G], f32r, tag="tp")
                nc.tensor.transpose(out=tp[:, :],
                                    in_=og_tok[:, ft * 128:(ft + 1) * 128],
                                    identity=e.id_fr[0:64, 0:64])
                nc.scalar.copy(out=og_feat[:, ft, :], in_=tp[:, :])

            wres = None
            wres = wpool.tile([128, FT, H], f32r, tag="wres")
            for kt in range(FT):
                nc.sync.dma_start(out=wres[:, kt, :],
                                 in_=e.wo[li, kt * 128:(kt + 1) * 128, :])
            og_w = lay.tile([64, H], f32r, tag="og_w")
            for half in range(2):
                hs = slice(half * 384, (half + 1) * 384)
                acc = P3.tile([64, 384], f32, tag="accW")
                for kt in range(FT):
                    nc.tensor.matmul(acc[:, :], og_feat[:, kt, :], wres[:, kt, hs],
                                     start=(kt == 0), stop=(kt == FT - 1))
                nc.scalar.copy(out=og_w[:, hs], in_=acc[:, :])

            for mt in range(FT):
                acc = P3.tile([128, T], f32, tag="acc")
                for kt in range(FT):
                    nc.tensor.matmul(acc[:, :],
                                     wres[:, kt, mt * 128:(mt + 1) * 128],
                                     out_attn[:, kt, :], start=(kt == 0),
                                     stop=False, skip_group_check=True)
                nc.tensor.matmul(acc[:, :], og_w[:, mt * 128:(mt + 1) * 128],
                                 e.sgt_sb[:, :], start=False, stop=True,
                                 skip_group_check=True)
                nc.vector.tensor_scalar_add(out=z_sb[:, mt, :], in0=acc[:, :],
                                            scalar1=bo_sb[:, mt:mt + 1])
                nc.vector.tensor_add(out=z_sb[:, mt, :], in0=z_sb[:, mt, :],
                                     in1=e.x_ext[:, mt, W:W + T])

            _layernorm(e, z_sb, x_mid, None, ln1s_sb, ln1b_sb, scl, lay, P3)

        # ==================== P4: FFN
        z2 = lay.tile([128, FT, T], f32r, tag="q_z")
        with tc.tile_pool(name=f"P4_{li}", bufs=1, space="PSUM") as P4:
            y2 = P4.tile([128, FT, T], f32, tag="y2")
            for ot in range(FF // 128):
                w1s = wff.tile([128, FT, 128], f32r, tag="w1s")
                nc.sync.dma_start(
                    out=w1s[:, :, :],
                    in_=e.w1[li, :, ot * 128:(ot + 1) * 128].rearrange(
                        "(k p) o -> p k o", p=128))
                hps = P4.tile([128, T], f32, tag="h")
                for kt in range(FT):
                    nc.tensor.matmul(hps[:, :], w1s[:, kt, :], x_mid[:, kt, :],
                                     start=(kt == 0), stop=(kt == FT - 1))
                h_sb = pp.tile([128, T], f32r, tag="h_sb")
                nc.scalar.activation(out=h_sb[:, :], in_=hps[:, :], func=Act.Gelu,
                                     bias=b1_sb[:, ot:ot + 1], scale=1.0)
                w2s = wff.tile([128, H], f32r, tag="w2s")
                nc.sync.dma_start(out=w2s[:, :],
                                 in_=e.w2[li, ot * 128:(ot + 1) * 128, :])
                for mt in range(FT):
                    nc.tensor.matmul(y2[:, mt, :],
                                     w2s[:, mt * 128:(mt + 1) * 128],
                                     h_sb[:, :], start=(ot == 0),
                                     stop=(ot == FF // 128 - 1),
                                     skip_group_check=True)
            for mt in range(FT):
                nc.vector.tensor_scalar_add(out=z2[:, mt, :], in0=y2[:, mt, :],
                                            scalar1=b2_sb[:, mt:mt + 1])
                nc.vector.tensor_add(out=z2[:, mt, :], in0=z2[:, mt, :],
                                     in1=x_mid[:, mt, :])

        # ==================== P5: LN2 (writes x_ext own), edges + xg collectives
        with tc.tile_pool(name=f"P5_{li}", bufs=1, space="PSUM") as P5:
            _layernorm(e, z2, None, OWN, ln2s_sb, ln2b_sb, scl, lay, P5)

            if li < e.n_layers - 1:
                for side, sl in ((0, slice(W, 2 * W)), (1, slice(T, W + T))):
                    nc.gpsimd.dma_start(out=e.edges_d[side], in_=e.x_ext[:, :, sl])
                e.gath8_d = e.dram.tile([NCORE * 1536, W], bf16,
                                        tag=f"g8_{li}", addr_space="Shared")
                nc.gpsimd.collective_compute(
                    "AllGather", Alu.bypass, replica_groups=AG8,
                    ins=[e.edges_d.opt()], outs=[e.gath8_d.opt()])
                x_tok = lay.tile([128, 4, H], f32r, tag="scr12")
                for j in range(4):
                    for ft in range(FT):
                        tp = P5.tile([128, 128], f32r, tag="tp")
                        nc.tensor.transpose(
                            out=tp[:, :],
                            in_=e.x_ext[:, ft, W + j * 128:W + (j + 1) * 128],
                            identity=e.id_fr)
                        nc.scalar.copy(out=x_tok[:, j, ft * 128:(ft + 1) * 128],
                                       in_=tp[:, :])
                xgc_sb = pp.tile([G, H], f32r, tag="gh64")
                for half in range(2):
                    hs = slice(half * 384, (half + 1) * 384)
                    acc = P5.tile([64, 384], f32, tag="accW")
                    for kt in range(4):
                        nc.tensor.matmul(acc[:, :], e.ssel_sb[:, kt, :],
                                         x_tok[:, kt, hs], start=(kt == 0),
                                         stop=(kt == 3))
                    nc.scalar.copy(out=xgc_sb[:, hs], in_=acc[:, :])
                nc.sync.dma_start(out=e.xgc_d[:, :], in_=xgc_sb[:, :])
                e.xgg_d = e.dram.tile([NCORE * G, H], f32r, tag=f"xgg{li}",
                                      addr_space="Shared")
                nc.gpsimd.collective_compute(
                    "AllGather", Alu.bypass, replica_groups=AG8,
                    ins=[e.xgc_d.opt()], outs=[e.xgg_d.opt()])


def _layernorm(e, z_sb, out, own_slice, s_t, b_t, scl, lay, P):
    """LN over features (feature-major). out=None -> write x_ext[:, :, own]."""
    nc = e.nc
    s1 = P.tile([1, T], f32, tag="stat1")
    for kt in range(FT):
        nc.tensor.matmul(s1[:, :], e.ones_sb, z_sb[:, kt, :],
                         start=(kt == 0), stop=(kt == FT - 1))
    s2 = P.tile([1, T], f32, tag="stat2")
    for kt in range(FT):
        zsq = lay.tile([128, T], f32r, tag="zsq")
        nc.scalar.activation(out=zsq[:, :], in_=z_sb[:, kt, :],
                             func=Act.Square, bias=0.0, scale=1.0)
        nc.tensor.matmul(s2[:, :], e.ones_sb, zsq[:, :],
                         start=(kt == 0), stop=(kt == FT - 1))
    A = scl.tile([1, T], f32, tag="lnA")   # mean
    B = scl.tile([1, T], f32, tag="lnB")   # msq -> var -> sd -> rstd
    C = scl.tile([1, T], f32, tag="lnC")   # mean^2 -> mrs
    nc.vector.tensor_scalar_mul(out=A[:, :], in0=s1[:, :], scalar1=1.0 / H)
    nc.vector.tensor_scalar_mul(out=B[:, :], in0=s2[:, :], scalar1=1.0 / H)
    nc.vector.tensor_tensor(out=C[:, :], in0=A[:, :], in1=A[:, :], op=Alu.mult)
    nc.vector.tensor_tensor(out=B[:, :], in0=B[:, :], in1=C[:, :], op=Alu.subtract)
    nc.scalar.activation(out=B[:, :], in_=B[:, :], func=Act.Sqrt,
                         bias=e.eps_sb[:, :], scale=1.0)
    nc.vector.reciprocal(out=B[:, :], in_=B[:, :])
    nc.vector.tensor_tensor(out=C[:, :], in0=A[:, :], in1=B[:, :], op=Alu.mult)
    nc.sync.dma_start(out=e.stats_d[0, :], in_=B[:, :])
    nc.sync.dma_start(out=e.stats_d[1, :], in_=C[:, :])
    rstd_b = lay.tile([128, T], f32, tag="ln_rstdb")
    mrs_b = lay.tile([128, T], f32, tag="ln_mrsb")
    nc.sync.dma_start(out=rstd_b[:, :], in_=_bcast(e.stats_d[0, :], 128))
    nc.sync.dma_start(out=mrs_b[:, :], in_=_bcast(e.stats_d[1, :], 128))
    for mt in range(FT):
        dst = e.x_ext[:, mt, own_slice] if out is None else out[:, mt, :]
        nc.vector.tensor_tensor(out=dst, in0=z_sb[:, mt, :], in1=rstd_b[:, :],
                                op=Alu.mult)
        nc.vector.tensor_tensor(out=dst, in0=dst, in1=mrs_b[:, :],
                                op=Alu.subtract)
        nc.vector.tensor_scalar(out=dst, in0=dst, scalar1=s_t[:, mt:mt + 1],
                                scalar2=b_t[:, mt:mt + 1], op0=Alu.mult,
                                op1=Alu.add)


# ---------------------------------------------------------------- driver

_CACHE = {}


def _get_program():
    if N_LAYERS not in _CACHE:
        _CACHE[N_LAYERS] = build_program(N_LAYERS)
    return _CACHE[N_LAYERS]


def kernel(**inputs):
    per_core, host = host_prep(inputs)
    nc = _get_program()

    src = {'wq': 'Wq', 'wk': 'Wk', 'wv': 'Wv', 'wqg': 'Wqg', 'wkg': 'Wkg',
           'wvg': 'Wvg', 'wo': 'Wo', 'w1': 'W1', 'w2': 'W2', 'bq': 'bq',
           'bk': 'bk', 'bv': 'bv', 'bqg': 'bqg', 'bkg': 'bkg', 'bvg': 'bvg',
           'bo': 'bo', 'b1': 'b1', 'b2': 'b2', 'ln1s': 'ln1_s', 'ln1b': 'ln1_b',
           'ln2s': 'ln2_s', 'ln2b': 'ln2_b'}
    shared = {k: np.ascontiguousarray(np.asarray(inputs[v], np.float32))
              for k, v in src.items()}

    in_maps = []
    for c in range(NCORE):
        m = dict(shared)
        d = per_core[c]
        for k in ('x0_ext', 'xg0_tok', 'maskadd', 'amadd', 'ssel', 'sgt',
                  'zmaskc', 'offs'):
            m[k] = d[k]
        import ml_dtypes
        cfr = np.zeros((128, 129), np.float32)
        cfr[:, :128] = np.eye(128, dtype=np.float32)
        cfr[:, 128] = 1.0
        m['cfr'] = cfr
        m['cbf'] = np.eye(128).astype(ml_dtypes.bfloat16)
        m['eps'] = np.full((1, 1), 1e-5, np.float32)
        in_maps.append(m)

    trace = bool(int(os.environ.get("KERNEL_TRACE", "0")))
    res = run_bass_kernel_spmd(nc, in_maps, core_ids=list(range(NCORE)),
                               trace=trace)
    kernel.last_result = res

    x = np.zeros((L, H), np.float32)
    for c in range(NCORE):
        xo = res.results[c]['xout']
        x[c * T:(c + 1) * T] = xo.transpose(2, 0, 1).reshape(T, H)
    kernel.last_x = x

    ids = host['ids'][0]
    cand_mask = ids == int(np.asarray(inputs['cand_token_id']))
    order = np.argsort(np.where(cand_mask, 0, 1).astype(np.int32), kind='stable')
    positions = order[:CMAX]
    valid = cand_mask[positions]
    g = x[positions]
    hh = _np_gelu(g @ np.asarray(inputs['Wh1'], np.float32)
                  + np.asarray(inputs['bh1'], np.float32))
    logits = (hh @ np.asarray(inputs['Wh2'], np.float32)
              + np.asarray(inputs['bh2'], np.float32))[:, 0]
    return logits[None].astype(np.float32), valid[None]


if __name__ == '__main__':
    print("building program ...")
    build_program(N_LAYERS)
    print("build OK")



# revision 22
# speedup vs baseline: 1.0930x; 1.0930x over previous
"""Trainium2 Bass kernel for nn_CrossEncoderLongformer (6-layer Longformer
cross-encoder, L=4096, H=768, 12 heads, W=256 sliding window, 64 global
tokens, B=1).

Sequence-sharded SPMD over 8 NeuronCores (512 tokens/core), feature-major
activations, per-layer halo AllGathers + flash-combined global rows.
Self-contained: host does embedding gather, LN_emb and the ranking head.
"""
import contextlib
import math
import os
import sys

if '/opt/trn_rl_repo' not in sys.path:
    sys.path.insert(0, '/opt/trn_rl_repo')

import numpy as np

import concourse.bass as bass
import concourse.bacc as bacc
import concourse.tile as tile
from concourse import mybir
from concourse.bass_utils import run_bass_kernel_spmd

H, NH, NL, FF, W, CMAX, VOC, L, G = 768, 12, 6, 3072, 256, 32, 50272, 4096, 64
DH = H // NH
SCALE = 1.0 / math.sqrt(DH)
NCORE = 8
T = L // NCORE                # 512
NBLK = T // W                 # 2
FT = H // 128                 # 6
TEXT = T + 2 * W              # 1024
SK = 3 * W + G                # 832 score columns
NEG = -1e9
OOB = 1 << 28

f32 = mybir.dt.float32
f32r = mybir.dt.float32r
bf16 = mybir.dt.bfloat16
i32 = mybir.dt.int32
Alu = mybir.AluOpType
Act = mybir.ActivationFunctionType
AX = mybir.AxisListType

N_LAYERS = int(os.environ.get("KERNEL_LAYERS", str(NL)))

AGA = [[0, 1], [2, 3], [4, 5], [6, 7]]
AGB = [[0, 7], [1, 2], [3, 4], [5, 6]]
AG8 = [list(range(NCORE))]


# ---------------------------------------------------------------- host side

def _np_ln(x, s, b, eps=1e-5):
    m = x.mean(-1, keepdims=True)
    v = ((x - m) ** 2).mean(-1, keepdims=True)
    return (x - m) / np.sqrt(v + eps) * s + b


def _np_gelu(x):
    try:
        from scipy.special import erf
        return 0.5 * x * (1.0 + erf(x / math.sqrt(2.0)))
    except Exception:
        e = np.vectorize(math.erf)
        return 0.5 * x * (1.0 + e(x / math.sqrt(2.0)))


def _featpack(x):
    """[N, H] -> [FT, 128, N]."""
    return np.ascontiguousarray(x.T.reshape(FT, 128, -1))


def host_prep(inputs):
    ids = np.asarray(inputs['input_ids'])
    am = np.asarray(inputs['attention_mask'])[0].astype(bool)
    gpos = np.asarray(inputs['global_positions']).astype(np.int64)
    emb = (np.asarray(inputs['emb_tok'])[ids[0]]
           + np.asarray(inputs['emb_pos'])[:L]).astype(np.float32)
    x0 = _np_ln(emb, np.asarray(inputs['ln_emb_s']), np.asarray(inputs['ln_emb_b']))

    is_glob = np.zeros(L, bool)
    is_glob[gpos] = True

    rel = np.arange(3 * W)[None, :] - W - np.arange(W)[:, None]
    band = np.abs(rel) <= W

    last_slot = {}
    for g, p in enumerate(gpos):
        last_slot[int(p)] = g

    x0p = np.pad(x0, ((W, W), (0, 0)))
    per_core = []
    import ml_dtypes
    for c in range(NCORE):
        d = {}
        xe = _featpack(x0p[c * T: c * T + TEXT]).astype(np.float32)
        d['x0_ext'] = xe
        d['x0b'] = xe.astype(ml_dtypes.bfloat16)

        mask_add = np.zeros((NBLK, 2, 128, SK), np.float32)
        for b in range(NBLK):
            gb = c * NBLK + b
            pos = gb * W + np.arange(3 * W) - W
            inb = (pos >= 0) & (pos < L)
            safe = np.clip(pos, 0, L - 1)
            key_ok = inb & (~is_glob[safe]) & am[safe]
            mloc = np.where(key_ok[None, :] & band, 0.0, NEG / SCALE)
            for qt in range(2):
                mask_add[b, qt, :, :3 * W] = mloc[qt * 128:(qt + 1) * 128]
        import ml_dtypes
        d['maskadd'] = mask_add.astype(ml_dtypes.bfloat16)

        amrow = np.where(am[c * T:(c + 1) * T], 0.0, NEG / SCALE).astype(np.float32)
        d['amadd'] = np.broadcast_to(amrow, (64, T)).copy()

        S = np.zeros((T, G), np.float32)
        for g, p in enumerate(gpos):
            p = int(p)
            if p // T == c:
                S[p % T, g] = 1.0
        d['ssel'] = np.ascontiguousarray(S.reshape(4, 128, G))

        SgT = np.zeros((G, T), np.float32)
        for p, g in last_slot.items():
            if p // T == c:
                SgT[g, p % T] = 1.0
        d['sgt'] = SgT

        zm = np.ones(T, np.float32)
        for p in gpos:
            p = int(p)
            if p // T == c:
                zm[p % T] = 0.0
        d['zmaskc'] = np.ascontiguousarray(zm.reshape(4, 128).T)

        # halo receive offsets into gath8 [8*1536, W]:
        # row = nbr*1536 + side*768 + p*6 + ft
        offs = np.zeros((128, 2, FT), np.int32)
        p_ar = np.arange(128)
        for combo in range(2):
            if combo == 0:              # left halo <- left neighbor's right edge
                if c == 0:
                    continue            # junk row 0, keys masked by inb
                nbr, side = c - 1, 1
            else:                       # right halo <- right neighbor's left edge
                if c == NCORE - 1:
                    continue
                nbr, side = c + 1, 0
            for ft in range(FT):
                offs[:, combo, ft] = nbr * 1536 + side * 768 + p_ar * 6 + ft
        d['offs'] = offs
        d['xg0_tok'] = x0[gpos].astype(np.float32)
        per_core.append(d)

    return per_core, dict(ids=ids, am=am, gpos=gpos)


# ------------------------------------------------------------- the program

class Env:
    pass


def build_program(n_layers=N_LAYERS):
    nc = bacc.Bacc("TRN2", target_bir_lowering=False, debug=False,
                   enable_asserts=True, num_devices=NCORE)
    e = Env()
    e.nc = nc
    e.n_layers = n_layers

    def din(name, shape, dt=f32r):
        return nc.dram_tensor(name, list(shape), dt, kind="ExternalInput").ap()

    for n in ('wq', 'wk', 'wv', 'wqg', 'wkg', 'wvg', 'wo'):
        setattr(e, n, din(n, [NL, H, H], bf16))
    e.w1 = din('w1', [NL, H, FF], bf16)
    e.w2 = din('w2', [NL, FF, H], bf16)
    for n in ('bq', 'bk', 'bv', 'bqg', 'bkg', 'bvg', 'bo', 'b2',
              'ln1s', 'ln1b', 'ln2s', 'ln2b'):
        setattr(e, n, din(n, [NL, H], f32))
    e.b1 = din('b1', [NL, FF], f32)

    e.x0_ext = din('x0_ext', [FT, 128, TEXT])
    e.x0b = din('x0b', [FT, 128, TEXT], bf16)
    e.xg0_tok = din('xg0_tok', [G, H])
    e.maskadd = din('maskadd', [NBLK, 2, 128, SK], bf16)
    e.amadd = din('amadd', [64, T], f32)
    e.ssel_i = din('ssel', [4, 128, G])
    e.sgt_i = din('sgt', [G, T])
    e.zmaskc = din('zmaskc', [128, 4], f32)
    e.offs_i = din('offs', [128, 2, FT], i32)
    e.cfr_i = din('cfr', [128, 129], f32r)
    e.cbf_i = din('cbf', [128, 128], bf16)
    e.eps_i = din('eps', [1, 1], f32)

    e.xout = nc.dram_tensor('xout', [FT, 128, T], f32r, kind="ExternalOutput").ap()

    with tile.TileContext(nc) as tc:
        e.tc = tc
        with contextlib.ExitStack() as stack:
            pers = stack.enter_context(tc.tile_pool(name="pers", bufs=1))
            dram = stack.enter_context(tc.tile_pool(name="dram", bufs=1, space="DRAM"))
            e.pers = pers
            e.dram = dram

            e.x_ext = pers.tile([128, FT, T], f32r, tag="x_ext")
            e.xb = pers.tile([128, FT, TEXT], bf16, tag="xb")
            e.mask_sb = pers.tile([128, NBLK, 2, SK], bf16, tag="mask")
            e.am_sb = pers.tile([64, T], f32, tag="am")
            e.ssel_sb = pers.tile([128, 4, G], f32r, tag="ssel")
            e.sgt_sb = pers.tile([64, T], f32r, tag="sgt")
            e.zmc_sb = pers.tile([128, 4], f32, tag="zmc")
            e.offs_sb = pers.tile([128, 2, FT], i32, tag="offs")
            e.cfr = pers.tile([128, 129], f32r, tag="cfr")
            e.id_bf = pers.tile([128, 128], bf16, tag="idbf")
            e.eps_sb = pers.tile([1, 1], f32, tag="eps")
            e.xg_tok = pers.tile([G, H], f32r, tag="xg_tok")
            e.id_fr = e.cfr[:, 0:128]
            e.ones_sb = e.cfr[:, 128:129]

            nc.sync.dma_start(out=e.cfr[:, :], in_=e.cfr_i[:, :])
            nc.sync.dma_start(out=e.id_bf[:, :], in_=e.cbf_i[:, :])
            nc.sync.dma_start(out=e.eps_sb[:, :], in_=e.eps_i[:, :])

            for ft in range(FT):
                nc.sync.dma_start(out=e.x_ext[:, ft, :], in_=e.x0_ext[ft][:, W:W + T])
                nc.sync.dma_start(out=e.xb[:, ft, :], in_=e.x0b[ft])
            for b in range(NBLK):
                for qt in range(2):
                    nc.sync.dma_start(out=e.mask_sb[:, b, qt, :], in_=e.maskadd[b, qt])
            nc.sync.dma_start(out=e.am_sb[:, :], in_=e.amadd[:, :])
            for kt in range(4):
                nc.sync.dma_start(out=e.ssel_sb[:, kt, :], in_=e.ssel_i[kt])
            nc.sync.dma_start(out=e.sgt_sb[:, :], in_=e.sgt_i[:, :])
            nc.sync.dma_start(out=e.zmc_sb[:, :], in_=e.zmaskc[:, :])
            nc.sync.dma_start(out=e.offs_sb[:, :, :], in_=e.offs_i[:, :, :])
            nc.sync.dma_start(out=e.xg_tok[:, :], in_=e.xg0_tok[:, :])

            e.edges_d = dram.tile([2, 128, FT, W], bf16, tag="edges")
            pass  # gath8 allocated per layer
            e.xgc_d = dram.tile([G, H], f32r, tag="xgc")
            pass  # xgg allocated per layer
            e.pc_d = dram.tile([G, H + 24], f32, tag="pc")
            pass  # pg allocated per layer
            e.stats_d = dram.tile([2, T], f32, tag="statsd")

            for li in range(n_layers):
                with nc.named_scope(f"layer{li}"):
                    _layer(e, li)

            for ft in range(FT):
                nc.sync.dma_start(out=e.xout[ft], in_=e.x_ext[:, ft, :])

    nc.compile()
    return nc


def _bcast(ap, n):
    """Broadcast an AP along a new leading (partition) axis of size n."""
    return bass.AP(tensor=ap.tensor, offset=ap.offset, ap=[[0, n]] + list(ap.ap))


def _layer(e, li):
    nc, tc = e.nc, e.tc
    OWN = slice(W, W + T)

    with contextlib.ExitStack() as ctx:
        lay = ctx.enter_context(tc.tile_pool(name=f"lay{li}", bufs=1))
        wpool = ctx.enter_context(tc.tile_pool(name=f"w{li}", bufs=1))
        wff = ctx.enter_context(tc.tile_pool(name=f"wff{li}", bufs=2))
        sc = ctx.enter_context(tc.tile_pool(name=f"sc{li}", bufs=2))
        scl = ctx.enter_context(tc.tile_pool(name=f"scl{li}", bufs=1))
        pp = ctx.enter_context(tc.tile_pool(name=f"pp{li}", bufs=2))

        # ---- per-layer bias / ln param tiles
        def bias_tile(src, cols, tag):
            t = lay.tile([128, cols], f32, tag=tag)
            nc.sync.dma_start(out=t[:, :],
                             in_=src[li].rearrange("(f p) -> p f", p=128))
            return t

        bq_sb = bias_tile(e.bq, FT, "bq")
        bk_sb = bias_tile(e.bk, FT, "bk")
        bqg_sb = bias_tile(e.bqg, FT, "bqg")
        bkg_sb = bias_tile(e.bkg, FT, "bkg")
        bo_sb = bias_tile(e.bo, FT, "bo")
        b2_sb = bias_tile(e.b2, FT, "b2")
        b1_sb = bias_tile(e.b1, FF // 128, "b1")
        ln1s_sb = bias_tile(e.ln1s, FT, "ln1s")
        ln1b_sb = bias_tile(e.ln1b, FT, "ln1b")
        ln2s_sb = bias_tile(e.ln2s, FT, "ln2s")
        ln2b_sb = bias_tile(e.ln2b, FT, "ln2b")

        bv_exp = lay.tile([128, H], f32, tag="bvexp")
        nc.sync.dma_start(out=bv_exp[:, :], in_=_bcast(e.bv[li], 128))
        bvg_exp = lay.tile([128, H], f32, tag="bvgexp")
        nc.sync.dma_start(out=bvg_exp[:, :], in_=_bcast(e.bvg[li], 128))

        # ==================== P1: receive, global projections, partials, qkv
        with tc.tile_pool(name=f"P1_{li}", bufs=1, space="PSUM") as P1:
            def p1(shape, tag, bufs_tag=None):
                return P1.tile(shape, f32, tag=tag)

            if li > 0:
                for combo, sl in enumerate([slice(0, W), slice(W + T, TEXT)]):
                    for ft in range(FT):
                        nc.gpsimd.indirect_dma_start(
                            out=e.xb[:, ft, sl],
                            out_offset=None,
                            in_=e.gath8_d[:, :],
                            in_offset=bass.IndirectOffsetOnAxis(
                                ap=e.offs_sb[:, combo, ft:ft + 1], axis=0),
                        )
                for c in range(NCORE):
                    xgch = pp.tile([G, H], f32r, tag="gh64")
                    nc.sync.dma_start(out=xgch[:, :],
                                     in_=e.xgg_d[c * G:(c + 1) * G, :])
                    if c == 0:
                        nc.vector.tensor_copy(out=e.xg_tok[:, :], in_=xgch[:, :])
                    else:
                        nc.vector.tensor_add(out=e.xg_tok[:, :],
                                             in0=e.xg_tok[:, :], in1=xgch[:, :])

            xg_bf = lay.tile([G, H], bf16, tag="xg_bf")
            nc.vector.tensor_copy(out=xg_bf[:, :], in_=e.xg_tok[:, :])
            xg_feat = lay.tile([128, FT, G], bf16, tag="xg_feat")
            for ft in range(FT):
                tp = P1.tile([128, G], bf16, tag="tp")
                nc.tensor.transpose(out=tp[:, :],
                                    in_=xg_bf[:, ft * 128:(ft + 1) * 128],
                                    identity=e.id_bf[0:64, 0:64])
                nc.scalar.copy(out=xg_feat[:, ft, :], in_=tp[:, :])

            def load_w(src):
                t = wpool.tile([128, FT, H], bf16, tag="wres")
                for kt in range(FT):
                    nc.sync.dma_start(out=t[:, kt, :],
                                     in_=src[li, kt * 128:(kt + 1) * 128, :])
                return t

            def proj_small(wres, bias, out):
                for ot in range(FT):
                    acc = P1.tile([128, G], f32, tag="acc")
                    for kt in range(FT):
                        nc.tensor.matmul(acc[:, :],
                                         wres[:, kt, ot * 128:(ot + 1) * 128],
                                         xg_feat[:, kt, :], start=(kt == 0),
                                         stop=(kt == FT - 1))
                    nc.vector.tensor_scalar_add(out=out[:, ot, :], in0=acc[:, :],
                                                scalar1=bias[:, ot:ot + 1])

            def proj_feat(wres, bias, out, src_cols, dst_cols, n):
                for ot in range(FT):
                    acc = P1.tile([128, 512], f32, tag="acc")
                    for kt in range(FT):
                        nc.tensor.matmul(acc[:, :n],
                                         wres[:, kt, ot * 128:(ot + 1) * 128],
                                         e.xb[:, kt, src_cols],
                                         start=(kt == 0), stop=(kt == FT - 1))
                    nc.vector.tensor_scalar_add(out=out[:, ot, dst_cols],
                                                in0=acc[:, :n],
                                                scalar1=bias[:, ot:ot + 1])

            def proj_tok(wres, bias_exp, out, tchunks, col0):
                for tc_ in tchunks:
                    for half in range(2):
                        hs = slice(half * 384, (half + 1) * 384)
                        acc = P1.tile([128, 384], f32, tag="acc")
                        cs = slice(col0 + tc_ * 128, col0 + (tc_ + 1) * 128)
                        for kt in range(FT):
                            nc.tensor.matmul(acc[:, :], e.xb[:, kt, cs],
                                             wres[:, kt, hs], start=(kt == 0),
                                             stop=(kt == FT - 1))
                        nc.vector.tensor_add(out=out[:, tc_, hs], in0=acc[:, :],
                                             in1=bias_exp[:, hs])

            # Wkg
            wres = load_w(e.wkg)
            kgs_feat = lay.tile([128, FT, G], bf16, tag="kgs")
            proj_small(wres, bkg_sb, kgs_feat)
            kg_own = lay.tile([128, FT, T], bf16, tag="kg_own")
            proj_feat(wres, bkg_sb, kg_own, OWN, slice(0, T), T)

            # Wvg
            wres = load_w(e.wvg)
            vgs_tok = lay.tile([64, H], bf16, tag="vgs")
            for half in range(2):
                hs = slice(half * 384, (half + 1) * 384)
                acc = P1.tile([64, 384], f32, tag="accW")
                for kt in range(FT):
                    nc.tensor.matmul(acc[:, :], xg_feat[:, kt, :], wres[:, kt, hs],
                                     start=(kt == 0), stop=(kt == FT - 1))
                nc.vector.tensor_add(out=vgs_tok[:, hs], in0=acc[:, :],
                                     in1=bvg_exp[0:64, hs])
            vg_own = lay.tile([128, 4, H], bf16, tag="vg_own")
            proj_tok(wres, bvg_exp, vg_own, range(4), W)

            # Wqg
            wres = load_w(e.wqg)
            qg_feat = lay.tile([128, FT, G], bf16, tag="qg")
            proj_small(wres, bqg_sb, qg_feat)

            # ---- flash partials + AllGather
            contrib = lay.tile([64, H + 24], f32, tag="contrib")
            for h in range(NH):
                hp, ht = (h % 2) * 64, h // 2
                sp = P1.tile([64, T], f32, tag="accW")
                nc.tensor.matmul(sp[:, :], qg_feat[hp:hp + 64, ht, :],
                                 kg_own[hp:hp + 64, ht, :], start=True, stop=True)
                s_sb = sc.tile([64, T], f32, tag="s_sb")
                nc.vector.tensor_add(out=s_sb[:, :], in0=sp[:, :], in1=e.am_sb[:, :])
                mx = sc.tile([64, 1], f32, tag="mx")
                nc.vector.reduce_max(out=mx[:, :], in_=s_sb[:, :], axis=AX.X)
                mneg = sc.tile([64, 1], f32, tag="mneg")
                nc.vector.tensor_scalar_mul(out=mneg[:, :], in0=mx[:, :],
                                            scalar1=-SCALE)
                pb = sc.tile([64, T], bf16, tag="p")
                lsum = sc.tile([64, 1], f32, tag="lsum")
                nc.scalar.activation(out=pb[:, :], in_=s_sb[:, :], func=Act.Exp,
                                     bias=mneg[:, :], scale=SCALE,
                                     accum_out=lsum[:, :])
                nc.vector.tensor_scalar_mul(out=contrib[:, H + h:H + h + 1],
                                            in0=mx[:, :], scalar1=SCALE)
                nc.vector.tensor_copy(out=contrib[:, H + 12 + h:H + 12 + h + 1],
                                      in_=lsum[:, :])
                pT = pp.tile([128, 4, 64], bf16, tag="pT")
                for j in range(4):
                    tp = P1.tile([128, 64], bf16, tag="tpb")
                    nc.tensor.transpose(out=tp[:, :],
                                        in_=pb[:, j * 128:(j + 1) * 128],
                                        identity=e.id_bf[:64, :64])
                    nc.scalar.copy(out=pT[:, j, :], in_=tp[:, :])
                oc = P1.tile([64, 64], f32, tag="accW")
                for j in range(4):
                    nc.tensor.matmul(oc[:, :], pT[:, j, :],
                                     vg_own[:, j, h * 64:(h + 1) * 64],
                                     start=(j == 0), stop=(j == 3))
                nc.scalar.copy(out=contrib[:, h * 64:(h + 1) * 64], in_=oc[:, :])
            nc.sync.dma_start(out=e.pc_d[:, :], in_=contrib[:, :])
            e.pg_d = e.dram.tile([NCORE * G, H + 24], f32, tag=f"pg{li}",
                                 addr_space="Shared")
            nc.gpsimd.collective_compute(
                "AllGather", Alu.bypass, replica_groups=AG8,
                ins=[e.pc_d.opt()], outs=[e.pg_d.opt()])

            # ---- own q/k/v
            wres = load_w(e.wq)
            q_sb = lay.tile([128, FT, T], bf16, tag="q_z")
            proj_feat(wres, bq_sb, q_sb, OWN, slice(0, T), T)

            wres = load_w(e.wk)
            k_ext = lay.tile([128, FT, TEXT], bf16, tag="k_ext")
            proj_feat(wres, bk_sb, k_ext, OWN, slice(W, W + T), T)
            proj_feat(wres, bk_sb, k_ext, slice(0, W), slice(0, W), W)
            proj_feat(wres, bk_sb, k_ext, slice(W + T, TEXT), slice(W + T, TEXT), W)

            wres = load_w(e.wv)
            v_tok = lay.tile([128, 8, H], bf16, tag="v_tok")
            proj_tok(wres, bv_exp, v_tok, [2, 3, 4, 5, 0, 1, 6, 7], 0)

        # ==================== P2: local attention
        out_attn = lay.tile([128, FT, T], bf16, tag="attnb")
        with tc.tile_pool(name=f"P2_{li}", bufs=1, space="PSUM") as P2:
            for b in range(NBLK):
                for h in range(NH):
                    hp, ht = (h % 2) * 64, h // 2
                    pboth = []
                    for qt in range(2):
                        qs = slice(b * W + qt * 128, b * W + (qt + 1) * 128)
                        spsum = P2.tile([128, SK], f32, tag=f"score{qt}")
                        nc.tensor.matmul(spsum[:, 0:512],
                                         q_sb[hp:hp + 64, ht, qs],
                                         k_ext[hp:hp + 64, ht, b * W:b * W + 512],
                                         start=True, stop=True)
                        nc.tensor.matmul(spsum[:, 512:3 * W],
                                         q_sb[hp:hp + 64, ht, qs],
                                         k_ext[hp:hp + 64, ht,
                                               b * W + 512:b * W + 3 * W],
                                         start=True, stop=True)
                        nc.tensor.matmul(spsum[:, 3 * W:SK],
                                         q_sb[hp:hp + 64, ht, qs],
                                         kgs_feat[hp:hp + 64, ht, :],
                                         start=True, stop=True)
                        s_sb = sc.tile([128, SK], f32, tag="s_sb")
                        nc.vector.tensor_add(out=s_sb[:, :], in0=spsum[:, :],
                                             in1=e.mask_sb[:, b, qt, :])
                        mx = sc.tile([128, 1], f32, tag="mx")
                        nc.vector.reduce_max(out=mx[:, :], in_=s_sb[:, :], axis=AX.X)
                        mneg = sc.tile([128, 1], f32, tag="mneg")
                        nc.vector.tensor_scalar_mul(out=mneg[:, :], in0=mx[:, :],
                                                    scalar1=-SCALE)
                        pb = sc.tile([128, SK], bf16, tag="p")
                        den = sc.tile([128, 1], f32, tag="lsum")
                        nc.scalar.activation(out=pb[:, :], in_=s_sb[:, :],
                                             func=Act.Exp, bias=mneg[:, :],
                                             scale=SCALE, accum_out=den[:, :])
                        rec = sc.tile([128, 1], f32, tag="rec")
                        nc.vector.reciprocal(out=rec[:, :], in_=den[:, :])
                        rz = sc.tile([128, 1], f32, tag="rz")
                        nc.vector.tensor_tensor(
                            out=rz[:, :], in0=rec[:, :],
                            in1=e.zmc_sb[:, b * 2 + qt:b * 2 + qt + 1], op=Alu.mult)
                        nc.vector.tensor_scalar_mul(out=pb[:, :], in0=pb[:, :],
                                                    scalar1=rz[:, :])
                        pboth.append(pb)
                    ov = P2.tile([64, 2 * 128], f32, tag="ov")
                    for j in range(6):
                        aT = P2.tile([128, 2 * 128], bf16, tag="aT")
                        aTsb = pp.tile([128, 2 * 128], bf16, tag="aT_sb")
                        for qt in range(2):
                            nc.tensor.transpose(
                                out=aT[:, qt * 128:(qt + 1) * 128],
                                in_=pboth[qt][:, j * 128:(j + 1) * 128],
                                identity=e.id_bf[:, :])
                        nc.scalar.copy(out=aTsb[:, :], in_=aT[:, :])
                        nc.tensor.matmul(ov[:, :],
                                         v_tok[:, 2 * b + j, h * 64:(h + 1) * 64],
                                         aTsb[:, :], start=(j == 0), stop=False,
                                         skip_group_check=True)
                    aTg = P2.tile([64, 2 * 128], bf16, tag="aTg")
                    aTgsb = pp.tile([64, 2 * 128], bf16, tag="aTg_sb")
                    for qt in range(2):
                        nc.tensor.transpose(out=aTg[:, qt * 128:(qt + 1) * 128],
                                            in_=pboth[qt][:, 3 * W:SK],
                                            identity=e.id_bf[:, :])
                    nc.scalar.copy(out=aTgsb[:, :], in_=aTg[:, :])
                    nc.tensor.matmul(ov[:, :], vgs_tok[:, h * 64:(h + 1) * 64],
                                     aTgsb[:, :], start=False, stop=True,
                                     skip_group_check=True)
                    nc.scalar.copy(out=out_attn[hp:hp + 64, ht, b * W:(b + 1) * W],
                                   in_=ov[:, :])

        # ==================== P3: combine partials, og, Wo, LN1
        z_sb = lay.tile([128, FT, T], f32r, tag="q_z")
        x_mid = lay.tile([128, FT, T], f32r, tag="x_mid")
        xb_mid = lay.tile([128, FT, T], bf16, tag="xb_mid")
        with tc.tile_pool(name=f"P3_{li}", bufs=1, space="PSUM") as P3:
            stat_all = lay.tile([64, NCORE, 24], f32, tag="stat_all")
            for c in range(NCORE):
                nc.sync.dma_start(out=stat_all[:, c, :],
                                 in_=e.pg_d[c * G:(c + 1) * G, H:H + 24])
            mg = scl.tile([64, 12], f32, tag="mg")
            nc.vector.tensor_copy(out=mg[:, :], in_=stat_all[:, 0, 0:12])
            for c in range(1, NCORE):
                nc.vector.tensor_tensor(out=mg[:, :], in0=mg[:, :],
                                        in1=stat_all[:, c, 0:12], op=Alu.max)
            wts = lay.tile([64, NCORE, 12], f32, tag="wts")
            lsum_g = scl.tile([64, 12], f32, tag="lsum_g")
            for c in range(NCORE):
                al = scl.tile([64, 12], f32, tag="alpha")
                nc.vector.tensor_tensor(out=al[:, :], in0=stat_all[:, c, 0:12],
                                        in1=mg[:, :], op=Alu.subtract)
                nc.scalar.activation(out=wts[:, c, :], in_=al[:, :], func=Act.Exp,
                                     bias=0.0, scale=1.0)
                lw = scl.tile([64, 12], f32, tag="lw")
                nc.vector.tensor_tensor(out=lw[:, :], in0=wts[:, c, :],
                                        in1=stat_all[:, c, 12:24], op=Alu.mult)
                if c == 0:
                    nc.vector.tensor_copy(out=lsum_g[:, :], in_=lw[:, :])
                else:
                    nc.vector.tensor_add(out=lsum_g[:, :], in0=lsum_g[:, :],
                                         in1=lw[:, :])
            rl = scl.tile([64, 12], f32, tag="rl")
            nc.vector.reciprocal(out=rl[:, :], in_=lsum_g[:, :])
            og_tok = lay.tile([64, H], f32r, tag="og_tok")
            for c in range(NCORE):
                wc = scl.tile([64, 12], f32, tag="wc")
                nc.vector.tensor_tensor(out=wc[:, :], in0=wts[:, c, :],
                                        in1=rl[:, :], op=Alu.mult)
                och = pp.tile([64, H], f32, tag="gh64")
                nc.sync.dma_start(out=och[:, :], in_=e.pg_d[c * G:(c + 1) * G, 0:H])
                for hh in range(NH):
                    nc.vector.tensor_scalar_mul(
                        out=och[:, hh * 64:(hh + 1) * 64],
                        in0=och[:, hh * 64:(hh + 1) * 64],
                        scalar1=wc[:, hh:hh + 1])
                if c == 0:
                    nc.vector.tensor_copy(out=og_tok[:, :], in_=och[:, :])
                else:
                    nc.vector.tensor_add(out=og_tok[:, :], in0=og_tok[:, :],
                                         in1=och[:, :])
            og_bf = lay.tile([64, H], bf16, tag="og_bf")
            nc.vector.tensor_copy(out=og_bf[:, :], in_=og_tok[:, :])
            og_feat = lay.tile([128, FT, G], bf16, tag="og_feat")
            for ft in range(FT):
                tp = P3.tile([128, G], bf16, tag="tp")
                nc.tensor.transpose(out=tp[:, :],
                                    in_=og_bf[:, ft * 128:(ft + 1) * 128],
                                    identity=e.id_bf[0:64, 0:64])
                nc.scalar.copy(out=og_feat[:, ft, :], in_=tp[:, :])

            wres = None
            wres = wpool.tile([128, FT, H], bf16, tag="wres")
            for kt in range(FT):
                nc.sync.dma_start(out=wres[:, kt, :],
                                 in_=e.wo[li, kt * 128:(kt + 1) * 128, :])
            og_w = lay.tile([64, H], f32r, tag="og_w")
            for half in range(2):
                hs = slice(half * 384, (half + 1) * 384)
                acc = P3.tile([64, 384], f32, tag="accW")
                for kt in range(FT):
                    nc.tensor.matmul(acc[:, :], og_feat[:, kt, :], wres[:, kt, hs],
                                     start=(kt == 0), stop=(kt == FT - 1))
                nc.scalar.copy(out=og_w[:, hs], in_=acc[:, :])

            for mt in range(FT):
                acc = P3.tile([128, T], f32, tag="acc")
                for kt in range(FT):
                    nc.tensor.matmul(acc[:, :],
                                     wres[:, kt, mt * 128:(mt + 1) * 128],
                                     out_attn[:, kt, :], start=(kt == 0),
                                     stop=False, skip_group_check=True)
                nc.tensor.matmul(acc[:, :], og_w[:, mt * 128:(mt + 1) * 128],
                                 e.sgt_sb[:, :], start=False, stop=True,
                                 skip_group_check=True)
                nc.vector.tensor_scalar_add(out=z_sb[:, mt, :], in0=acc[:, :],
                                            scalar1=bo_sb[:, mt:mt + 1])
                nc.vector.tensor_add(out=z_sb[:, mt, :], in0=z_sb[:, mt, :],
                                     in1=e.x_ext[:, mt, :])

            _layernorm(e, z_sb, x_mid, None, ln1s_sb, ln1b_sb, scl, lay, P3,
                       bf_out=xb_mid)

        # ==================== P4: FFN
        z2 = lay.tile([128, FT, T], f32r, tag="q_z")
        with tc.tile_pool(name=f"P4_{li}", bufs=1, space="PSUM") as P4:
            y2 = P4.tile([128, FT, T], f32, tag="y2")
            for ot in range(FF // 128):
                w1s = wff.tile([128, FT, 128], bf16, tag="w1s")
                nc.sync.dma_start(
                    out=w1s[:, :, :],
                    in_=e.w1[li, :, ot * 128:(ot + 1) * 128].rearrange(
                        "(k p) o -> p k o", p=128))
                hps = P4.tile([128, T], f32, tag="h")
                for kt in range(FT):
                    nc.tensor.matmul(hps[:, :], w1s[:, kt, :], xb_mid[:, kt, :],
                                     start=(kt == 0), stop=(kt == FT - 1))
                h_sb = pp.tile([128, T], bf16, tag="h_sb")
                nc.scalar.activation(out=h_sb[:, :], in_=hps[:, :], func=Act.Gelu,
                                     bias=b1_sb[:, ot:ot + 1], scale=1.0)
                w2s = wff.tile([128, H], bf16, tag="w2s")
                nc.sync.dma_start(out=w2s[:, :],
                                 in_=e.w2[li, ot * 128:(ot + 1) * 128, :])
                for mt in range(FT):
                    nc.tensor.matmul(y2[:, mt, :],
                                     w2s[:, mt * 128:(mt + 1) * 128],
                                     h_sb[:, :], start=(ot == 0),
                                     stop=(ot == FF // 128 - 1),
                                     skip_group_check=True)
            for mt in range(FT):
                nc.vector.tensor_scalar_add(out=z2[:, mt, :], in0=y2[:, mt, :],
                                            scalar1=b2_sb[:, mt:mt + 1])
                nc.vector.tensor_add(out=z2[:, mt, :], in0=z2[:, mt, :],
                                     in1=x_mid[:, mt, :])

        # ==================== P5: LN2 (writes x_ext own), edges + xg collectives
        with tc.tile_pool(name=f"P5_{li}", bufs=1, space="PSUM") as P5:
            _layernorm(e, z2, None, slice(0, T), ln2s_sb, ln2b_sb, scl, lay,
                       P5, bf_out=e.xb, bf_cols=OWN)

            if li < e.n_layers - 1:
                for side, sl in ((0, slice(W, 2 * W)), (1, slice(T, W + T))):
                    nc.gpsimd.dma_start(out=e.edges_d[side], in_=e.xb[:, :, sl])
                e.gath8_d = e.dram.tile([NCORE * 1536, W], bf16,
                                        tag=f"g8_{li}", addr_space="Shared")
                nc.gpsimd.collective_compute(
                    "AllGather", Alu.bypass, replica_groups=AG8,
                    ins=[e.edges_d.opt()], outs=[e.gath8_d.opt()])
                x_tok = lay.tile([128, 4, H], f32r, tag="scr12")
                for j in range(4):
                    for ft in range(FT):
                        tp = P5.tile([128, 128], f32r, tag="tp")
                        nc.tensor.transpose(
                            out=tp[:, :],
                            in_=e.x_ext[:, ft, j * 128:(j + 1) * 128],
                            identity=e.id_fr)
                        nc.scalar.copy(out=x_tok[:, j, ft * 128:(ft + 1) * 128],
                                       in_=tp[:, :])
                xgc_sb = pp.tile([G, H], f32r, tag="gh64")
                for half in range(2):
                    hs = slice(half * 384, (half + 1) * 384)
                    acc = P5.tile([64, 384], f32, tag="accW")
                    for kt in range(4):
                        nc.tensor.matmul(acc[:, :], e.ssel_sb[:, kt, :],
                                         x_tok[:, kt, hs], start=(kt == 0),
                                         stop=(kt == 3))
                    nc.scalar.copy(out=xgc_sb[:, hs], in_=acc[:, :])
                nc.sync.dma_start(out=e.xgc_d[:, :], in_=xgc_sb[:, :])
                e.xgg_d = e.dram.tile([NCORE * G, H], f32r, tag=f"xgg{li}",
                                      addr_space="Shared")
                nc.gpsimd.collective_compute(
                    "AllGather", Alu.bypass, replica_groups=AG8,
                    ins=[e.xgc_d.opt()], outs=[e.xgg_d.opt()])


def _layernorm(e, z_sb, out, own_slice, s_t, b_t, scl, lay, P, bf_out=None,
               bf_cols=slice(None)):
    """LN over features (feature-major). out=None -> write x_ext[:, :, own]."""
    nc = e.nc
    s1 = P.tile([1, T], f32, tag="stat1")
    for kt in range(FT):
        nc.tensor.matmul(s1[:, :], e.ones_sb, z_sb[:, kt, :],
                         start=(kt == 0), stop=(kt == FT - 1))
    s2 = P.tile([1, T], f32, tag="stat2")
    for kt in range(FT):
        zsq = lay.tile([128, T], f32r, tag="zsq")
        nc.scalar.activation(out=zsq[:, :], in_=z_sb[:, kt, :],
                             func=Act.Square, bias=0.0, scale=1.0)
        nc.tensor.matmul(s2[:, :], e.ones_sb, zsq[:, :],
                         start=(kt == 0), stop=(kt == FT - 1))
    A = scl.tile([1, T], f32, tag="lnA")   # mean
    B = scl.tile([1, T], f32, tag="lnB")   # msq -> var -> sd -> rstd
    C = scl.tile([1, T], f32, tag="lnC")   # mean^2 -> mrs
    nc.vector.tensor_scalar_mul(out=A[:, :], in0=s1[:, :], scalar1=1.0 / H)
    nc.vector.tensor_scalar_mul(out=B[:, :], in0=s2[:, :], scalar1=1.0 / H)
    nc.vector.tensor_tensor(out=C[:, :], in0=A[:, :], in1=A[:, :], op=Alu.mult)
    nc.vector.tensor_tensor(out=B[:, :], in0=B[:, :], in1=C[:, :], op=Alu.subtract)
    nc.scalar.activation(out=B[:, :], in_=B[:, :], func=Act.Sqrt,
                         bias=e.eps_sb[:, :], scale=1.0)
    nc.vector.reciprocal(out=B[:, :], in_=B[:, :])
    nc.vector.tensor_tensor(out=C[:, :], in0=A[:, :], in1=B[:, :], op=Alu.mult)
    nc.sync.dma_start(out=e.stats_d[0, :], in_=B[:, :])
    nc.sync.dma_start(out=e.stats_d[1, :], in_=C[:, :])
    rstd_b = lay.tile([128, T], f32, tag="ln_rstdb")
    mrs_b = lay.tile([128, T], f32, tag="ln_mrsb")
    nc.sync.dma_start(out=rstd_b[:, :], in_=_bcast(e.stats_d[0, :], 128))
    nc.sync.dma_start(out=mrs_b[:, :], in_=_bcast(e.stats_d[1, :], 128))
    for mt in range(FT):
        dst = e.x_ext[:, mt, own_slice] if out is None else out[:, mt, :]
        nc.vector.tensor_tensor(out=dst, in0=z_sb[:, mt, :], in1=rstd_b[:, :],
                                op=Alu.mult)
        nc.vector.tensor_tensor(out=dst, in0=dst, in1=mrs_b[:, :],
                                op=Alu.subtract)
        nc.vector.tensor_scalar(out=dst, in0=dst, scalar1=s_t[:, mt:mt + 1],
                                scalar2=b_t[:, mt:mt + 1], op0=Alu.mult,
                                op1=Alu.add)
        if bf_out is not None:
            nc.scalar.copy(out=bf_out[:, mt, bf_cols], in_=dst)


# ---------------------------------------------------------------- driver

_CACHE = {}


def _get_program():
    if N_LAYERS not in _CACHE:
        _CACHE[N_LAYERS] = build_program(N_LAYERS)
    return _CACHE[N_LAYERS]


def kernel(**inputs):
    per_core, host = host_prep(inputs)
    nc = _get_program()

    import ml_dtypes
    wsrc = {'wq': 'Wq', 'wk': 'Wk', 'wv': 'Wv', 'wqg': 'Wqg', 'wkg': 'Wkg',
            'wvg': 'Wvg', 'wo': 'Wo', 'w1': 'W1', 'w2': 'W2'}
    bsrc = {'bq': 'bq', 'bk': 'bk', 'bv': 'bv', 'bqg': 'bqg', 'bkg': 'bkg',
            'bvg': 'bvg', 'bo': 'bo', 'b1': 'b1', 'b2': 'b2',
            'ln1s': 'ln1_s', 'ln1b': 'ln1_b', 'ln2s': 'ln2_s', 'ln2b': 'ln2_b'}
    shared = {k: np.ascontiguousarray(
                  np.asarray(inputs[v], np.float32).astype(ml_dtypes.bfloat16))
              for k, v in wsrc.items()}
    shared.update({k: np.ascontiguousarray(np.asarray(inputs[v], np.float32))
                   for k, v in bsrc.items()})

    in_maps = []
    for c in range(NCORE):
        m = dict(shared)
        d = per_core[c]
        for k in ('x0_ext', 'x0b', 'xg0_tok', 'maskadd', 'amadd', 'ssel', 'sgt',
                  'zmaskc', 'offs'):
            m[k] = d[k]
        import ml_dtypes
        cfr = np.zeros((128, 129), np.float32)
        cfr[:, :128] = np.eye(128, dtype=np.float32)
        cfr[:, 128] = 1.0
        m['cfr'] = cfr
        m['cbf'] = np.eye(128).astype(ml_dtypes.bfloat16)
        m['eps'] = np.full((1, 1), 1e-5, np.float32)
        in_maps.append(m)

    trace = bool(int(os.environ.get("KERNEL_TRACE", "0")))
    res = run_bass_kernel_spmd(nc, in_maps, core_ids=list(range(NCORE)),
                               trace=trace)
    kernel.last_result = res

    x = np.zeros((L, H), np.float32)
    for c in range(NCORE):
        xo = res.results[c]['xout']
        x[c * T:(c + 1) * T] = xo.transpose(2, 0, 1).reshape(T, H)
    kernel.last_x = x

    ids = host['ids'][0]
    cand_mask = ids == int(np.asarray(inputs['cand_token_id']))
    order = np.argsort(np.where(cand_mask, 0, 1).astype(np.int32), kind='stable')
    positions = order[:CMAX]
    valid = cand_mask[positions]
    g = x[positions]
    hh = _np_gelu(g @ np.asarray(inputs['Wh1'], np.float32)
                  + np.asarray(inputs['bh1'], np.float32))
    logits = (hh @ np.asarray(inputs['Wh2'], np.float32)
              + np.asarray(inputs['bh2'], np.float32))[:, 0]
    return logits[None].astype(np.float32), valid[None]


if __name__ == '__main__':
    print("building program ...")
    build_program(N_LAYERS)
    print("build OK")



# revision 27
# speedup vs baseline: 1.5332x; 1.4027x over previous
"""Trainium2 Bass kernel for nn_CrossEncoderLongformer (6-layer Longformer
cross-encoder, L=4096, H=768, 12 heads, W=256 sliding window, 64 global
tokens, B=1).

Sequence-sharded SPMD over 8 NeuronCores (512 tokens/core), feature-major
activations, bf16 weights/streams with fp32 residual/accumulation.
Attention is computed key-major (S^T = K^T.T @ Q directly from the PE) with
max-free softmax (scores are provably small), multiplicative band masks,
denominators via a ones-column appended to V, and normalization folded into
a broadcast-matmul + one multiply per feature tile. Global-token rows use
max-free flash partials combined with a single AllReduce.
Self-contained: host does embedding gather, LN_emb and the ranking head.
"""
import contextlib
import math
import os
import sys

if '/opt/trn_rl_repo' not in sys.path:
    sys.path.insert(0, '/opt/trn_rl_repo')

import numpy as np

import concourse.bass as bass
import concourse.bacc as bacc
import concourse.tile as tile
from concourse import mybir
from concourse.bass_utils import run_bass_kernel_spmd

H, NH, NL, FF, W, CMAX, VOC, L, G = 768, 12, 6, 3072, 256, 32, 50272, 4096, 64
DH = H // NH
SCALE = 1.0 / math.sqrt(DH)
NCORE = 8
T = L // NCORE                # 512
NBLK = T // W                 # 2
FT = H // 128                 # 6
TEXT = T + 2 * W              # 1024
OOB = 1 << 28

f32 = mybir.dt.float32
f32r = mybir.dt.float32r
bf16 = mybir.dt.bfloat16
i32 = mybir.dt.int32
Alu = mybir.AluOpType
Act = mybir.ActivationFunctionType
AX = mybir.AxisListType

N_LAYERS = int(os.environ.get("KERNEL_LAYERS", str(NL)))

AG8 = [list(range(NCORE))]


# ---------------------------------------------------------------- host side

def _np_ln(x, s, b, eps=1e-5):
    m = x.mean(-1, keepdims=True)
    v = ((x - m) ** 2).mean(-1, keepdims=True)
    return (x - m) / np.sqrt(v + eps) * s + b


def _np_gelu(x):
    try:
        from scipy.special import erf
        return 0.5 * x * (1.0 + erf(x / math.sqrt(2.0)))
    except Exception:
        e = np.vectorize(math.erf)
        return 0.5 * x * (1.0 + e(x / math.sqrt(2.0)))


def _featpack(x):
    """[N, H] -> [FT, 128, N]."""
    return np.ascontiguousarray(x.T.reshape(FT, 128, -1))


def host_prep(inputs):
    import ml_dtypes
    ids = np.asarray(inputs['input_ids'])
    am = np.asarray(inputs['attention_mask'])[0].astype(bool)
    gpos = np.asarray(inputs['global_positions']).astype(np.int64)
    emb = (np.asarray(inputs['emb_tok'])[ids[0]]
           + np.asarray(inputs['emb_pos'])[:L]).astype(np.float32)
    x0 = _np_ln(emb, np.asarray(inputs['ln_emb_s']), np.asarray(inputs['ln_emb_b']))

    is_glob = np.zeros(L, bool)
    is_glob[gpos] = True

    last_slot = {}
    for g, p in enumerate(gpos):
        last_slot[int(p)] = g

    x0p = np.pad(x0, ((W, W), (0, 0)))
    per_core = []
    for c in range(NCORE):
        d = {}
        xe = _featpack(x0p[c * T: c * T + TEXT]).astype(np.float32)
        d['x0_ext'] = np.ascontiguousarray(xe[:, :, W:W + T])
        d['x0b'] = xe.astype(ml_dtypes.bfloat16)
        d['xg0f'] = _featpack(x0[gpos]).astype(ml_dtypes.bfloat16)

        # key-major multiplicative masks: band & key_ok, [NBLK, 6, 128, W]
        band01 = np.zeros((NBLK, 6, 128, W), np.float32)
        for b in range(NBLK):
            gb = c * NBLK + b
            pos = gb * W + np.arange(3 * W) - W
            inb = (pos >= 0) & (pos < L)
            safe = np.clip(pos, 0, L - 1)
            key_ok = inb & (~is_glob[safe]) & am[safe]          # [3W]
            rel = np.arange(3 * W)[:, None] - W - np.arange(W)[None, :]
            m = (np.abs(rel) <= W) & key_ok[:, None]
            band01[b] = m.reshape(6, 128, W)
        d['band01'] = band01.astype(ml_dtypes.bfloat16)

        # attention_mask as 0/1 per own key, [128, 4] (key tile j, partition p)
        amk = am[c * T:(c + 1) * T].astype(np.float32).reshape(4, 128).T
        d['am01k'] = np.ascontiguousarray(amk)

        # zero-rows (global query positions) as 0/1, [2, FT, T] bf16
        zm = np.ones(T, np.float32)
        for p in gpos:
            p = int(p)
            if p // T == c:
                zm[p % T] = 0.0
        d['zrow2'] = np.ascontiguousarray(
            np.broadcast_to(zm[None, None, :], (2, FT, T))).astype(
            ml_dtypes.bfloat16)

        S = np.zeros((T, G), np.float32)
        for g, p in enumerate(gpos):
            p = int(p)
            if p // T == c:
                S[p % T, g] = 1.0
        d['ssel'] = np.ascontiguousarray(S.reshape(4, 128, G)).astype(
            ml_dtypes.bfloat16)

        SgT = np.zeros((G, T), np.float32)
        for p, g in last_slot.items():
            if p // T == c:
                SgT[g, p % T] = 1.0
        d['sgt'] = SgT

        # halo receive offsets into gath8 [8*1536, W]:
        # row = nbr*1536 + side*768 + p*6 + ft
        offs = np.full((128, 2, FT), OOB, np.int32)
        p_ar = np.arange(128)
        for combo in range(2):
            if combo == 0:              # left halo <- left neighbor's right edge
                if c == 0:
                    continue            # keys masked by inb
                nbr, side = c - 1, 1
            else:                       # right halo <- right neighbor's left edge
                if c == NCORE - 1:
                    continue
                nbr, side = c + 1, 0
            for ft in range(FT):
                offs[:, combo, ft] = nbr * 1536 + side * 768 + p_ar * 6 + ft
        # core 0 / 7 one-sided: keep junk reads in-bounds (row 0)
        offs[offs == OOB] = 0
        d['offs'] = offs
        per_core.append(d)

    return per_core, dict(ids=ids, am=am, gpos=gpos)


# ------------------------------------------------------------- the program

class Env:
    pass


def build_program(n_layers=N_LAYERS):
    nc = bacc.Bacc("TRN2", target_bir_lowering=False, debug=False,
                   enable_asserts=True, num_devices=NCORE)
    e = Env()
    e.nc = nc
    e.n_layers = n_layers

    def din(name, shape, dt=f32r):
        return nc.dram_tensor(name, list(shape), dt, kind="ExternalInput").ap()

    for n in ('wq', 'wk', 'wv', 'wqg', 'wkg', 'wvg', 'wo'):
        setattr(e, n, din(n, [NL, H, H], bf16))
    e.w1 = din('w1', [NL, H, FF], bf16)
    e.w2 = din('w2', [NL, FF, H], bf16)
    for n in ('bq', 'bk', 'bv', 'bqg', 'bkg', 'bvg', 'bo', 'b2',
              'ln1s', 'ln1b', 'ln2s', 'ln2b'):
        setattr(e, n, din(n, [NL, H], f32))
    e.b1 = din('b1', [NL, FF], f32)

    e.x0_ext = din('x0_ext', [FT, 128, T])
    e.x0b = din('x0b', [FT, 128, TEXT], bf16)
    e.xg0f_i = din('xg0f', [FT, 128, G], bf16)
    e.band01_i = din('band01', [NBLK, 6, 128, W], bf16)
    e.am01_i = din('am01k', [128, 4], f32)
    e.zrow_i = din('zrow2', [2, FT, T], bf16)
    e.ssel_i = din('ssel', [4, 128, G], bf16)
    e.sgt_i = din('sgt', [G, T])
    e.offs_i = din('offs', [128, 2, FT], i32)
    e.selh_i = din('selh', [2, 128], f32r)
    e.onesr_i = din('onesr', [1, 128], f32r)
    e.cons_i = din('cons', [128, 1], f32r)
    e.cbf_i = din('cbf', [128, 128], bf16)
    e.eps_i = din('eps', [1, 1], f32)

    e.xout = nc.dram_tensor('xout', [FT, 128, T], f32r, kind="ExternalOutput").ap()

    with tile.TileContext(nc) as tc:
        e.tc = tc
        with contextlib.ExitStack() as stack:
            pers = stack.enter_context(tc.tile_pool(name="pers", bufs=1))
            dram = stack.enter_context(tc.tile_pool(name="dram", bufs=1, space="DRAM"))
            e.pers = pers
            e.dram = dram

            e.x_ext = pers.tile([128, FT, T], f32r, tag="x_ext")
            e.xb = pers.tile([128, FT, TEXT], bf16, tag="xb")
            e.band_sb = pers.tile([128, NBLK, 6, W], bf16, tag="band")
            e.am01_sb = pers.tile([128, 4], f32, tag="am01")
            e.zrow_sb = pers.tile([2, FT, T], bf16, tag="zrow")
            e.ssel_sb = pers.tile([128, 4, G], bf16, tag="ssel")
            e.sgt_sb = pers.tile([64, T], f32r, tag="sgt")
            e.offs_sb = pers.tile([128, 2, FT], i32, tag="offs")
            e.selh_sb = pers.tile([2, 128], f32r, tag="selh")
            e.onesr_sb = pers.tile([1, 128], f32r, tag="onesr")
            e.ones_sb = pers.tile([128, 1], f32r, tag="ones")
            e.id_bf = pers.tile([128, 128], bf16, tag="idbf")
            e.eps_sb = pers.tile([1, 1], f32, tag="eps")

            nc.sync.dma_start(out=e.selh_sb[:, :], in_=e.selh_i[:, :])
            nc.sync.dma_start(out=e.onesr_sb[:, :], in_=e.onesr_i[:, :])
            nc.sync.dma_start(out=e.ones_sb[:, :], in_=e.cons_i[:, :])
            nc.sync.dma_start(out=e.id_bf[:, :], in_=e.cbf_i[:, :])
            nc.sync.dma_start(out=e.eps_sb[:, :], in_=e.eps_i[:, :])

            for ft in range(FT):
                nc.sync.dma_start(out=e.x_ext[:, ft, :], in_=e.x0_ext[ft])
                nc.sync.dma_start(out=e.xb[:, ft, :], in_=e.x0b[ft])
            for b in range(NBLK):
                for j in range(6):
                    nc.sync.dma_start(out=e.band_sb[:, b, j, :],
                                     in_=e.band01_i[b, j])
            nc.sync.dma_start(out=e.am01_sb[:, :], in_=e.am01_i[:, :])
            nc.sync.dma_start(out=e.zrow_sb[:, :, :], in_=e.zrow_i[:, :, :])
            for kt in range(4):
                nc.sync.dma_start(out=e.ssel_sb[:, kt, :], in_=e.ssel_i[kt])
            nc.sync.dma_start(out=e.sgt_sb[:, :], in_=e.sgt_i[:, :])
            nc.sync.dma_start(out=e.offs_sb[:, :, :], in_=e.offs_i[:, :, :])

            e.edges_d = dram.tile([2, 128, FT, W], bf16, tag="edges")
            e.pc_d = dram.tile([H + 12, G], f32, tag="pc")
            e.xgc_d = dram.tile([H, G], f32, tag="xgc")

            for li in range(n_layers):
                with nc.named_scope(f"layer{li}"):
                    _layer(e, li)

            for ft in range(FT):
                nc.sync.dma_start(out=e.xout[ft], in_=e.x_ext[:, ft, :])

    nc.compile()
    return nc


def _bcast(ap, n):
    """Broadcast an AP along a new leading (partition) axis of size n."""
    return bass.AP(tensor=ap.tensor, offset=ap.offset, ap=[[0, n]] + list(ap.ap))


def _layer(e, li):
    nc, tc = e.nc, e.tc
    OWN = slice(W, W + T)

    with contextlib.ExitStack() as ctx:
        lay = ctx.enter_context(tc.tile_pool(name=f"lay{li}", bufs=1))
        wpool = ctx.enter_context(tc.tile_pool(name=f"w{li}", bufs=2))
        wff = ctx.enter_context(tc.tile_pool(name=f"wff{li}", bufs=2))
        sc = ctx.enter_context(tc.tile_pool(name=f"sc{li}", bufs=2))
        scl = ctx.enter_context(tc.tile_pool(name=f"scl{li}", bufs=1))
        pp = ctx.enter_context(tc.tile_pool(name=f"pp{li}", bufs=2))

        # ---- per-layer bias / ln param tiles
        def bias_tile(src, cols, tag):
            t = lay.tile([128, cols], f32, tag=tag)
            nc.sync.dma_start(out=t[:, :],
                             in_=src[li].rearrange("(f p) -> p f", p=128))
            return t

        bq_sb = bias_tile(e.bq, FT, "bq")
        bk_sb = bias_tile(e.bk, FT, "bk")
        bqg_sb = bias_tile(e.bqg, FT, "bqg")
        bkg_sb = bias_tile(e.bkg, FT, "bkg")
        bo_sb = bias_tile(e.bo, FT, "bo")
        b2_sb = bias_tile(e.b2, FT, "b2")
        b1_sb = bias_tile(e.b1, FF // 128, "b1")
        ln1s_sb = bias_tile(e.ln1s, FT, "ln1s")
        ln1b_sb = bias_tile(e.ln1b, FT, "ln1b")
        ln2s_sb = bias_tile(e.ln2s, FT, "ln2s")
        ln2b_sb = bias_tile(e.ln2b, FT, "ln2b")

        bv_exp = lay.tile([128, H], f32, tag="bvexp")
        nc.sync.dma_start(out=bv_exp[:, :], in_=_bcast(e.bv[li], 128))
        bvg_exp = lay.tile([128, H], f32, tag="bvgexp")
        nc.sync.dma_start(out=bvg_exp[:, :], in_=_bcast(e.bvg[li], 128))

        def load_w(src):
            t = wpool.tile([128, FT, H], bf16, tag="wres")
            for kt in range(FT):
                nc.sync.dma_start(out=t[:, kt, :],
                                 in_=src[li, kt * 128:(kt + 1) * 128, :])
            return t

        # ==================== P1: receive, projections, flash partials
        q_sb = lay.tile([128, FT, T], bf16, tag="q_z")
        k_ext = lay.tile([128, FT, TEXT], bf16, tag="k_ext")
        v_tok = lay.tile([128, 8, NH, DH + 2], bf16, tag="v_tok")
        vg2 = lay.tile([128, 4, NH, DH + 2], bf16, tag="vg2")
        vgs2 = lay.tile([64, NH, DH + 2], bf16, tag="vgs2")
        kg_own = lay.tile([128, FT, T], bf16, tag="kg_own")
        kgs_feat = lay.tile([128, FT, G], bf16, tag="kgs")
        qg_feat = lay.tile([128, FT, G], bf16, tag="qg")
        xg_feat = lay.tile([128, FT, G], bf16, tag="xg_feat")
        ctrb = lay.tile([128, FT, G], f32, tag="ctrb")
        cden = lay.tile([2, FT, G], f32, tag="cden")

        nc.vector.memset(v_tok[:, :, :, DH:DH + 2], 1.0)
        nc.vector.memset(vg2[:, :, :, DH:DH + 2], 1.0)
        nc.vector.memset(vgs2[:, :, DH:DH + 2], 1.0)

        with tc.tile_pool(name=f"P1_{li}", bufs=1, space="PSUM") as P1:
            # halo + global-token receive (DMA queues; wait on prev collectives)
            if li > 0:
                for combo, sl in enumerate([slice(0, W), slice(W + T, TEXT)]):
                    for ft in range(FT):
                        nc.gpsimd.indirect_dma_start(
                            out=e.xb[:, ft, sl],
                            out_offset=None,
                            in_=e.gath8_d[:, :],
                            in_offset=bass.IndirectOffsetOnAxis(
                                ap=e.offs_sb[:, combo, ft:ft + 1], axis=0),
                        )
                for ft in range(FT):
                    nc.gpsimd.dma_start(
                        out=xg_feat[:, ft, :],
                        in_=e.xgg_d[ft * 128:(ft + 1) * 128, :])
            else:
                for ft in range(FT):
                    nc.sync.dma_start(out=xg_feat[:, ft, :], in_=e.xg0f_i[ft])

            def proj_small(wres, bias, out):
                for ot in range(FT):
                    acc = P1.tile([128, G], f32, tag="accg")
                    for kt in range(FT):
                        nc.tensor.matmul(acc[:, :],
                                         wres[:, kt, ot * 128:(ot + 1) * 128],
                                         xg_feat[:, kt, :], start=(kt == 0),
                                         stop=(kt == FT - 1))
                    nc.vector.tensor_scalar_add(out=out[:, ot, :], in0=acc[:, :],
                                                scalar1=bias[:, ot:ot + 1])

            def proj_feat(wres, bias, out, src_cols, dst_cols, n):
                for ot in range(FT):
                    acc = P1.tile([128, 512], f32, tag="acc")
                    for kt in range(FT):
                        nc.tensor.matmul(acc[:, :n],
                                         wres[:, kt, ot * 128:(ot + 1) * 128],
                                         e.xb[:, kt, src_cols],
                                         start=(kt == 0), stop=(kt == FT - 1))
                    nc.vector.tensor_scalar_add(out=out[:, ot, dst_cols],
                                                in0=acc[:, :n],
                                                scalar1=bias[:, ot:ot + 1])

            def proj_tok(wres, bias_exp, out, tchunks, col0):
                # out: [128, nchunk, NH, DH+1]; writes the DH feature columns
                for tc_ in tchunks:
                    for half in range(2):
                        hs = slice(half * 384, (half + 1) * 384)
                        acc = P1.tile([128, 384], f32, tag="acc")
                        cs = slice(col0 + tc_ * 128, col0 + (tc_ + 1) * 128)
                        for kt in range(FT):
                            nc.tensor.matmul(acc[:, :], e.xb[:, kt, cs],
                                             wres[:, kt, hs], start=(kt == 0),
                                             stop=(kt == FT - 1))
                        nc.vector.tensor_add(
                            out=out[:, tc_, half * 6:(half + 1) * 6, 0:DH],
                            in0=acc[:, :], in1=bias_exp[:, hs])

            # 1. own-x projections (no external deps)
            wres = load_w(e.wq)
            proj_feat(wres, bq_sb, q_sb, OWN, slice(0, T), T)

            wres = load_w(e.wkg)
            proj_feat(wres, bkg_sb, kg_own, OWN, slice(0, T), T)
            proj_small(wres, bkg_sb, kgs_feat)

            wres = load_w(e.wvg)
            proj_tok(wres, bvg_exp, vg2, range(4), W)
            for half in range(2):
                hs = slice(half * 384, (half + 1) * 384)
                acc = P1.tile([64, 384], f32, tag="accW")
                for kt in range(FT):
                    nc.tensor.matmul(acc[:, :], xg_feat[:, kt, :], wres[:, kt, hs],
                                     start=(kt == 0), stop=(kt == FT - 1))
                nc.vector.tensor_add(
                    out=vgs2[:, half * 6:(half + 1) * 6, 0:DH],
                    in0=acc[:, :], in1=bvg_exp[0:64, hs])

            wres = load_w(e.wqg)
            proj_small(wres, bqg_sb, qg_feat)

            # 2. flash partials for global query rows (key-major, max-free)
            pfs = {}

            def flash_scores(h):
                hp, ht = (h % 2) * 64, h // 2
                fT = P1.tile([128, 4, G], f32, tag=f"fT{h % 2}")
                for j in range(4):
                    nc.tensor.matmul(fT[:, j, :],
                                     kg_own[hp:hp + 64, ht, j * 128:(j + 1) * 128],
                                     qg_feat[hp:hp + 64, ht, :],
                                     start=True, stop=True)
                return fT

            def flash_soft(h, fT):
                pf = sc.tile([128, 4, G], bf16, tag="pf")
                nc.scalar.activation(out=pf[:, :, :], in_=fT[:, :, :],
                                     func=Act.Exp, bias=0.0, scale=SCALE)
                for j in range(4):
                    nc.vector.tensor_scalar_mul(out=pf[:, j, :], in0=pf[:, j, :],
                                                scalar1=e.am01_sb[:, j:j + 1])
                pfs[h] = pf

            def flash_av(h):
                hp, ht = (h % 2) * 64, h // 2
                ovf = P1.tile([DH + 2, G], f32, tag=f"ovf{h % 2}")
                for j in range(4):
                    nc.tensor.matmul(ovf[:, :], vg2[:, j, h, :], pfs[h][:, j, :],
                                     start=(j == 0), stop=(j == 3),
                                     skip_group_check=True)
                nc.vector.tensor_copy(out=ctrb[hp:hp + 64, ht, :],
                                      in_=ovf[0:64, :])
                # odd head -> partitions 0:2 (slot 0 scratch), even -> 0:1
                if h % 2:
                    nc.scalar.copy(out=cden[0:2, ht, :], in_=ovf[DH:DH + 2, :])
                else:
                    nc.scalar.copy(out=cden[0:1, ht, :], in_=ovf[DH:DH + 1, :])

            horder = [1, 0, 3, 2, 5, 4, 7, 6, 9, 8, 11, 10]
            fT = flash_scores(horder[0])
            for hi, h in enumerate(horder):
                nf = flash_scores(horder[hi + 1]) if hi + 1 < NH else None
                flash_soft(h, fT)
                flash_av(h)
                fT = nf

            for ft in range(FT):
                nc.sync.dma_start(out=e.pc_d[ft * 128:(ft + 1) * 128, :],
                                 in_=ctrb[:, ft, :])
            nc.sync.dma_start(out=e.pc_d[H:H + 12, :], in_=cden[:, :, :])
            e.pg_d = e.dram.tile([H + 12, G], f32, tag=f"pg{li}",
                                 addr_space="Shared")
            nc.gpsimd.collective_compute(
                "AllReduce", Alu.add, replica_groups=AG8,
                ins=[e.pc_d.opt()], outs=[e.pg_d.opt()])

            # 3. k / v over own + halo tokens (halo newly received)
            wres = load_w(e.wv)
            proj_tok(wres, bv_exp, v_tok, [2, 3, 4, 5, 1, 6, 0, 7], 0)

            wres = load_w(e.wk)
            proj_feat(wres, bk_sb, k_ext, OWN, slice(W, W + T), T)
            proj_feat(wres, bk_sb, k_ext, slice(0, W), slice(0, W), W)
            proj_feat(wres, bk_sb, k_ext, slice(W + T, TEXT), slice(W + T, TEXT), W)

        # ==================== P2: local attention (key-major, pipelined)
        attn_raw = lay.tile([128, FT, T], bf16, tag="x_mid")
        out_attn = lay.tile([128, FT, T], bf16, tag="attnb")
        rz_all = lay.tile([2, FT, T], f32r, tag="rz_all")
        items = [(b, h) for b in range(NBLK)
         for h in (1, 0, 3, 2, 5, 4, 7, 6, 9, 8, 11, 10)]
        with tc.tile_pool(name=f"P2a_{li}", bufs=2, space="PSUM") as P2a, \
             tc.tile_pool(name=f"P2b_{li}", bufs=1, space="PSUM") as P2b:
            sTs, sgs, pTs, pTgs = {}, {}, {}, {}

            def p2_scores(i):
                b, h = items[i]
                hp, ht = (h % 2) * 64, h // 2
                qs = slice(b * W, (b + 1) * W)
                sT = P2a.tile([128, 6, W], f32, tag="sT")
                for j in range(6):
                    nc.tensor.matmul(
                        sT[:, j, :],
                        k_ext[hp:hp + 64, ht, b * W + j * 128:b * W + j * 128 + 128],
                        q_sb[hp:hp + 64, ht, qs], start=True, stop=True)
                sg = P2b.tile([64, W], f32, tag="sg")
                nc.tensor.matmul(sg[:, :], kgs_feat[hp:hp + 64, ht, :],
                                 q_sb[hp:hp + 64, ht, qs], start=True, stop=True)
                sTs[i], sgs[i] = sT, sg

            def p2_soft(i):
                b, h = items[i]
                pT = sc.tile([128, 6, W], bf16, tag="pT")
                nc.scalar.activation(out=pT[:, :, :], in_=sTs[i][:, :, :],
                                     func=Act.Exp, bias=0.0, scale=SCALE)
                nc.vector.tensor_tensor(out=pT[:, :, :], in0=pT[:, :, :],
                                        in1=e.band_sb[:, b, :, :], op=Alu.mult)
                pTg = sc.tile([64, W], bf16, tag="pTg")
                nc.scalar.activation(out=pTg[:, :], in_=sgs[i][:, :],
                                     func=Act.Exp, bias=0.0, scale=SCALE)
                pTs[i], pTgs[i] = pT, pTg

            def p2_av(i):
                b, h = items[i]
                hp, ht = (h % 2) * 64, h // 2
                qs = slice(b * W, (b + 1) * W)
                ov = P2b.tile([DH + 2, W], f32, tag="ov")
                for j in range(6):
                    nc.tensor.matmul(ov[:, :], v_tok[:, 2 * b + j, h, :],
                                     pTs[i][:, j, :], start=(j == 0), stop=False,
                                     skip_group_check=True)
                nc.tensor.matmul(ov[:, :], vgs2[:, h, :], pTgs[i][:, :],
                                 start=False, stop=True, skip_group_check=True)
                nc.vector.tensor_copy(out=attn_raw[hp:hp + 64, ht, qs],
                                      in_=ov[0:DH, :])
                if h % 2:
                    nc.scalar.copy(out=rz_all[0:2, ht, qs],
                                   in_=ov[DH:DH + 2, :])
                else:
                    nc.scalar.copy(out=rz_all[0:1, ht, qs],
                                   in_=ov[DH:DH + 1, :])

            p2_scores(0)
            for i in range(len(items)):
                if i + 1 < len(items):
                    p2_scores(i + 1)
                p2_soft(i)
                p2_av(i)

        # normalize: out = raw * bcast(zrow / denom)
        with tc.tile_pool(name=f"P2n_{li}", bufs=2, space="PSUM") as P2n:
            with nc.allow_low_precision(reason="f32r == f32 bits"):
                nc.vector.reciprocal(out=rz_all[:, :, :], in_=rz_all[:, :, :])
            nc.vector.tensor_tensor(out=rz_all[:, :, :], in0=rz_all[:, :, :],
                                    in1=e.zrow_sb[:, :, :], op=Alu.mult)
            for ft in range(FT):
                dx = P2n.tile([128, T], f32, tag="dexp")
                nc.tensor.matmul(dx[:, :], e.selh_sb[:, :], rz_all[:, ft, :],
                                 start=True, stop=True)
                nc.vector.tensor_tensor(out=out_attn[:, ft, :],
                                        in0=attn_raw[:, ft, :], in1=dx[:, :],
                                        op=Alu.mult)

        # ==================== P3: combine global rows, Wo, LN1
        z_sb = lay.tile([128, FT, T], f32r, tag="q_z")
        x_mid = lay.tile([128, FT, T], f32r, tag="x_mid")
        xb_mid = lay.tile([128, FT, T], bf16, tag="xb_mid")
        with tc.tile_pool(name=f"P3_{li}", bufs=1, space="PSUM") as P3:
            og_raw = lay.tile([128, FT, G], f32, tag="og_raw")
            for ft in range(FT):
                nc.sync.dma_start(out=og_raw[:, ft, :],
                                 in_=e.pg_d[ft * 128:(ft + 1) * 128, :])
            grz = scl.tile([2, FT, G], f32r, tag="grz")
            nc.gpsimd.dma_start(
                out=grz[:, :, :],
                in_=e.pg_d[H:H + 12, :].rearrange("(two f) g -> two f g", two=2))
            with nc.allow_low_precision(reason="f32r == f32 bits"):
                nc.vector.reciprocal(out=grz[:, :, :], in_=grz[:, :, :])
            og_feat = lay.tile([128, FT, G], bf16, tag="og_feat")
            for ft in range(FT):
                dg = P3.tile([128, G], f32, tag="dg")
                nc.tensor.matmul(dg[:, :], e.selh_sb[:, :], grz[:, ft, :],
                                 start=True, stop=True)
                nc.vector.tensor_tensor(out=og_feat[:, ft, :],
                                        in0=og_raw[:, ft, :], in1=dg[:, :],
                                        op=Alu.mult)

            wres = load_w(e.wo)
            og_w = lay.tile([64, H], f32r, tag="og_w")
            for half in range(2):
                hs = slice(half * 384, (half + 1) * 384)
                acc = P3.tile([64, 384], f32, tag="accW")
                for kt in range(FT):
                    nc.tensor.matmul(acc[:, :], og_feat[:, kt, :], wres[:, kt, hs],
                                     start=(kt == 0), stop=(kt == FT - 1))
                nc.scalar.copy(out=og_w[:, hs], in_=acc[:, :])

            for mt in range(FT):
                acc = P3.tile([128, T], f32, tag="acc")
                for kt in range(FT):
                    nc.tensor.matmul(acc[:, :],
                                     wres[:, kt, mt * 128:(mt + 1) * 128],
                                     out_attn[:, kt, :], start=(kt == 0),
                                     stop=False, skip_group_check=True)
                nc.tensor.matmul(acc[:, :], og_w[:, mt * 128:(mt + 1) * 128],
                                 e.sgt_sb[:, :], start=False, stop=True,
                                 skip_group_check=True)
                nc.vector.tensor_scalar_add(out=z_sb[:, mt, :], in0=acc[:, :],
                                            scalar1=bo_sb[:, mt:mt + 1])
                nc.vector.tensor_add(out=z_sb[:, mt, :], in0=z_sb[:, mt, :],
                                     in1=e.x_ext[:, mt, :])

            _layernorm(e, z_sb, x_mid, None, ln1s_sb, ln1b_sb, scl, lay, P3,
                       bf_out=xb_mid)

        # ==================== P4: FFN
        z2 = lay.tile([128, FT, T], f32r, tag="q_z")
        with tc.tile_pool(name=f"P4_{li}", bufs=1, space="PSUM") as P4:
            y2 = P4.tile([128, FT, T], f32, tag="y2")
            for ot in range(FF // 128):
                w1s = wff.tile([128, FT, 128], bf16, tag="w1s")
                nc.sync.dma_start(
                    out=w1s[:, :, :],
                    in_=e.w1[li, :, ot * 128:(ot + 1) * 128].rearrange(
                        "(k p) o -> p k o", p=128))
                hps = P4.tile([128, T], f32, tag=f"h{ot % 2}")
                for kt in range(FT):
                    nc.tensor.matmul(hps[:, :], w1s[:, kt, :], xb_mid[:, kt, :],
                                     start=(kt == 0), stop=(kt == FT - 1))
                h_sb = pp.tile([128, T], bf16, tag="h_sb")
                nc.scalar.activation(out=h_sb[:, :], in_=hps[:, :], func=Act.Gelu,
                                     bias=b1_sb[:, ot:ot + 1], scale=1.0)
                w2s = wff.tile([128, H], bf16, tag="w2s")
                nc.sync.dma_start(out=w2s[:, :],
                                 in_=e.w2[li, ot * 128:(ot + 1) * 128, :])
                for mt in range(FT):
                    nc.tensor.matmul(y2[:, mt, :],
                                     w2s[:, mt * 128:(mt + 1) * 128],
                                     h_sb[:, :], start=(ot == 0),
                                     stop=(ot == FF // 128 - 1),
                                     skip_group_check=True)
            for mt in range(FT):
                nc.vector.tensor_scalar_add(out=z2[:, mt, :], in0=y2[:, mt, :],
                                            scalar1=b2_sb[:, mt:mt + 1])
                nc.vector.tensor_add(out=z2[:, mt, :], in0=z2[:, mt, :],
                                     in1=x_mid[:, mt, :])

        # ==================== P5: LN2 (writes x/xb own), edges + xg collectives
        with tc.tile_pool(name=f"P5_{li}", bufs=1, space="PSUM") as P5:
            _layernorm(e, z2, None, slice(0, T), ln2s_sb, ln2b_sb, scl, lay,
                       P5, bf_out=e.xb, bf_cols=OWN)

            if li < e.n_layers - 1:
                for side, sl in ((0, slice(W, 2 * W)), (1, slice(T, W + T))):
                    nc.gpsimd.dma_start(out=e.edges_d[side], in_=e.xb[:, :, sl])
                e.gath8_d = e.dram.tile([NCORE * 1536, W], bf16,
                                        tag=f"g8_{li}", addr_space="Shared")
                nc.gpsimd.collective_compute(
                    "AllGather", Alu.bypass, replica_groups=AG8,
                    ins=[e.edges_d.opt()], outs=[e.gath8_d.opt()])

                x_tok = lay.tile([128, 4, H], bf16, tag="x_tok")
                for j in range(4):
                    for ft in range(FT):
                        tp = P5.tile([128, 128], bf16, tag="tp")
                        nc.tensor.transpose(
                            out=tp[:, :],
                            in_=e.xb[:, ft, W + j * 128:W + (j + 1) * 128],
                            identity=e.id_bf)
                        nc.scalar.copy(out=x_tok[:, j, ft * 128:(ft + 1) * 128],
                                       in_=tp[:, :])
                xgc_sb = lay.tile([128, FT, G], f32, tag="xgc_sb")
                for ft in range(FT):
                    xgp = P5.tile([128, G], f32, tag="xgp")
                    for j in range(4):
                        nc.tensor.matmul(xgp[:, :],
                                         x_tok[:, j, ft * 128:(ft + 1) * 128],
                                         e.ssel_sb[:, j, :], start=(j == 0),
                                         stop=(j == 3))
                    nc.scalar.copy(out=xgc_sb[:, ft, :], in_=xgp[:, :])
                for ft in range(FT):
                    nc.sync.dma_start(out=e.xgc_d[ft * 128:(ft + 1) * 128, :],
                                     in_=xgc_sb[:, ft, :])
                e.xgg_d = e.dram.tile([H, G], f32, tag=f"xgg{li}",
                                      addr_space="Shared")
                nc.gpsimd.collective_compute(
                    "AllReduce", Alu.add, replica_groups=AG8,
                    ins=[e.xgc_d.opt()], outs=[e.xgg_d.opt()])


def _layernorm(e, z_sb, out, own_slice, s_t, b_t, scl, lay, P, bf_out=None,
               bf_cols=slice(None)):
    """LN over features (feature-major). out=None -> write x_ext own."""
    nc = e.nc
    s1 = P.tile([1, T], f32, tag="stat1")
    for kt in range(FT):
        nc.tensor.matmul(s1[:, :], e.ones_sb, z_sb[:, kt, :],
                         start=(kt == 0), stop=(kt == FT - 1))
    s2 = P.tile([1, T], f32, tag="stat2")
    for kt in range(FT):
        zsq = lay.tile([128, T], f32r, tag="zsq")
        nc.scalar.activation(out=zsq[:, :], in_=z_sb[:, kt, :],
                             func=Act.Square, bias=0.0, scale=1.0)
        nc.tensor.matmul(s2[:, :], e.ones_sb, zsq[:, :],
                         start=(kt == 0), stop=(kt == FT - 1))
    A = scl.tile([1, T], f32r, tag="lnA")   # mean
    B = scl.tile([1, T], f32r, tag="lnB")   # msq -> var -> sd -> rstd
    C = scl.tile([1, T], f32r, tag="lnC")   # mean*rstd
    nc.vector.tensor_scalar_mul(out=A[:, :], in0=s1[:, :], scalar1=1.0 / H)
    nc.vector.tensor_scalar_mul(out=B[:, :], in0=s2[:, :], scalar1=1.0 / H)
    nc.vector.tensor_tensor(out=C[:, :], in0=A[:, :], in1=A[:, :], op=Alu.mult)
    nc.vector.tensor_tensor(out=B[:, :], in0=B[:, :], in1=C[:, :], op=Alu.subtract)
    nc.scalar.activation(out=B[:, :], in_=B[:, :], func=Act.Sqrt,
                         bias=e.eps_sb[:, :], scale=1.0)
    with nc.allow_low_precision(reason="f32r == f32 bits"):
        nc.vector.reciprocal(out=B[:, :], in_=B[:, :])
    nc.vector.tensor_tensor(out=C[:, :], in0=A[:, :], in1=B[:, :], op=Alu.mult)
    rstd_b = P.tile([128, T], f32, tag="lnbc1")
    nc.tensor.matmul(rstd_b[:, :], e.onesr_sb[:, :], B[:, :],
                     start=True, stop=True)
    mrs_b = P.tile([128, T], f32, tag="lnbc2")
    nc.tensor.matmul(mrs_b[:, :], e.onesr_sb[:, :], C[:, :],
                     start=True, stop=True)
    for mt in range(FT):
        dst = e.x_ext[:, mt, own_slice] if out is None else out[:, mt, :]
        nc.vector.tensor_tensor(out=dst, in0=z_sb[:, mt, :], in1=rstd_b[:, :],
                                op=Alu.mult)
        nc.vector.tensor_tensor(out=dst, in0=dst, in1=mrs_b[:, :],
                                op=Alu.subtract)
        nc.vector.tensor_scalar(out=dst, in0=dst, scalar1=s_t[:, mt:mt + 1],
                                scalar2=b_t[:, mt:mt + 1], op0=Alu.mult,
                                op1=Alu.add)
        if bf_out is not None:
            nc.scalar.copy(out=bf_out[:, mt, bf_cols], in_=dst)


# ---------------------------------------------------------------- driver

_CACHE = {}


def _get_program():
    if N_LAYERS not in _CACHE:
        _CACHE[N_LAYERS] = build_program(N_LAYERS)
    return _CACHE[N_LAYERS]


def kernel(**inputs):
    import ml_dtypes
    per_core, host = host_prep(inputs)
    nc = _get_program()

    wsrc = {'wq': 'Wq', 'wk': 'Wk', 'wv': 'Wv', 'wqg': 'Wqg', 'wkg': 'Wkg',
            'wvg': 'Wvg', 'wo': 'Wo', 'w1': 'W1', 'w2': 'W2'}
    bsrc = {'bq': 'bq', 'bk': 'bk', 'bv': 'bv', 'bqg': 'bqg', 'bkg': 'bkg',
            'bvg': 'bvg', 'bo': 'bo', 'b1': 'b1', 'b2': 'b2',
            'ln1s': 'ln1_s', 'ln1b': 'ln1_b', 'ln2s': 'ln2_s', 'ln2b': 'ln2_b'}
    shared = {k: np.ascontiguousarray(
                  np.asarray(inputs[v], np.float32).astype(ml_dtypes.bfloat16))
              for k, v in wsrc.items()}
    shared.update({k: np.ascontiguousarray(np.asarray(inputs[v], np.float32))
                   for k, v in bsrc.items()})

    selh = np.zeros((2, 128), np.float32)
    selh[0, 0:64] = 1.0
    selh[1, 64:128] = 1.0
    shared['selh'] = selh
    shared['onesr'] = np.ones((1, 128), np.float32)
    shared['cons'] = np.ones((128, 1), np.float32)
    shared['cbf'] = np.eye(128).astype(ml_dtypes.bfloat16)
    shared['eps'] = np.full((1, 1), 1e-5, np.float32)

    in_maps = []
    for c in range(NCORE):
        m = dict(shared)
        d = per_core[c]
        for k in ('x0_ext', 'x0b', 'xg0f', 'band01', 'am01k', 'zrow2', 'ssel',
                  'sgt', 'offs'):
            m[k] = d[k]
        in_maps.append(m)

    trace = bool(int(os.environ.get("KERNEL_TRACE", "0")))
    res = run_bass_kernel_spmd(nc, in_maps, core_ids=list(range(NCORE)),
                               trace=trace)
    kernel.last_result = res

    x = np.zeros((L, H), np.float32)
    for c in range(NCORE):
        xo = res.results[c]['xout']
        x[c * T:(c + 1) * T] = xo.transpose(2, 0, 1).reshape(T, H)
    kernel.last_x = x

    ids = host['ids'][0]
    cand_mask = ids == int(np.asarray(inputs['cand_token_id']))
    order = np.argsort(np.where(cand_mask, 0, 1).astype(np.int32), kind='stable')
    positions = order[:CMAX]
    valid = cand_mask[positions]
    g = x[positions]
    hh = _np_gelu(g @ np.asarray(inputs['Wh1'], np.float32)
                  + np.asarray(inputs['bh1'], np.float32))
    logits = (hh @ np.asarray(inputs['Wh2'], np.float32)
              + np.asarray(inputs['bh2'], np.float32))[:, 0]
    return logits[None].astype(np.float32), valid[None]


if __name__ == '__main__':
    print("building program ...")
    build_program(N_LAYERS)
    print("build OK")


# revision 31
# speedup vs baseline: 1.8448x; 1.2032x over previous
"""Trainium2 Bass kernel for nn_CrossEncoderLongformer (6-layer Longformer
cross-encoder, L=4096, H=768, 12 heads, W=256 sliding window, 64 global
tokens, B=1).

Sequence-sharded SPMD over 8 NeuronCores (512 tokens/core), feature-major
activations, bf16 weights/streams with fp32 residual/accumulation.
Attention is computed key-major (S^T = K^T.T @ Q directly from the PE) with
max-free softmax (scores are provably small), multiplicative band masks,
denominators via a ones-column appended to V, and normalization folded into
a broadcast-matmul + one multiply per feature tile. Global-token rows use
max-free flash partials combined with a single AllReduce.
Self-contained: host does embedding gather, LN_emb and the ranking head.
"""
import contextlib
import math
import os
import sys

if '/opt/trn_rl_repo' not in sys.path:
    sys.path.insert(0, '/opt/trn_rl_repo')

import numpy as np

import concourse.bass as bass
import concourse.bacc as bacc
import concourse.tile as tile
from concourse import mybir
from concourse.bass_utils import run_bass_kernel_spmd

H, NH, NL, FF, W, CMAX, VOC, L, G = 768, 12, 6, 3072, 256, 32, 50272, 4096, 64
DH = H // NH
SCALE = 1.0 / math.sqrt(DH)
NCORE = 8
T = L // NCORE                # 512
NBLK = T // W                 # 2
FT = H // 128                 # 6
TEXT = T + 2 * W              # 1024
OOB = 1 << 28

f32 = mybir.dt.float32
f32r = mybir.dt.float32r
bf16 = mybir.dt.bfloat16
i32 = mybir.dt.int32
Alu = mybir.AluOpType
Act = mybir.ActivationFunctionType
AX = mybir.AxisListType

N_LAYERS = int(os.environ.get("KERNEL_LAYERS", str(NL)))

AG8 = [list(range(NCORE))]


# ---------------------------------------------------------------- host side

def _np_ln(x, s, b, eps=1e-5):
    m = x.mean(-1, keepdims=True)
    v = ((x - m) ** 2).mean(-1, keepdims=True)
    return (x - m) / np.sqrt(v + eps) * s + b


def _np_gelu(x):
    try:
        from scipy.special import erf
        return 0.5 * x * (1.0 + erf(x / math.sqrt(2.0)))
    except Exception:
        e = np.vectorize(math.erf)
        return 0.5 * x * (1.0 + e(x / math.sqrt(2.0)))


def _featpack(x):
    """[N, H] -> [FT, 128, N]."""
    return np.ascontiguousarray(x.T.reshape(FT, 128, -1))


def host_prep(inputs):
    import ml_dtypes
    ids = np.asarray(inputs['input_ids'])
    am = np.asarray(inputs['attention_mask'])[0].astype(bool)
    gpos = np.asarray(inputs['global_positions']).astype(np.int64)
    emb = (np.asarray(inputs['emb_tok'])[ids[0]]
           + np.asarray(inputs['emb_pos'])[:L]).astype(np.float32)
    x0 = _np_ln(emb, np.asarray(inputs['ln_emb_s']), np.asarray(inputs['ln_emb_b']))

    is_glob = np.zeros(L, bool)
    is_glob[gpos] = True

    last_slot = {}
    for g, p in enumerate(gpos):
        last_slot[int(p)] = g

    x0p = np.pad(x0, ((W, W), (0, 0)))
    per_core = []
    for c in range(NCORE):
        d = {}
        xe = _featpack(x0p[c * T: c * T + TEXT]).astype(np.float32)
        d['x0_ext'] = np.ascontiguousarray(xe[:, :, W:W + T])
        d['x0b'] = xe.astype(ml_dtypes.bfloat16)
        d['xg0f'] = _featpack(x0[gpos]).astype(ml_dtypes.bfloat16)

        # key-major multiplicative masks: band & key_ok, [NBLK, 6, 128, W]
        band01 = np.zeros((NBLK, 6, 128, W), np.float32)
        for b in range(NBLK):
            gb = c * NBLK + b
            pos = gb * W + np.arange(3 * W) - W
            inb = (pos >= 0) & (pos < L)
            safe = np.clip(pos, 0, L - 1)
            key_ok = inb & (~is_glob[safe]) & am[safe]          # [3W]
            rel = np.arange(3 * W)[:, None] - W - np.arange(W)[None, :]
            m = (np.abs(rel) <= W) & key_ok[:, None]
            band01[b] = m.reshape(6, 128, W)
        d['band01'] = band01.astype(ml_dtypes.bfloat16)

        # attention_mask as 0/1 per own key, [128, 4] (key tile j, partition p)
        amk = am[c * T:(c + 1) * T].astype(np.float32).reshape(4, 128).T
        d['am01k'] = np.ascontiguousarray(amk)

        # zero-rows (global query positions) as 0/1, [2, FT, T] bf16
        zm = np.ones(T, np.float32)
        for p in gpos:
            p = int(p)
            if p // T == c:
                zm[p % T] = 0.0
        d['zrow2'] = np.ascontiguousarray(
            np.broadcast_to(zm[None, None, :], (2, FT, T))).astype(
            ml_dtypes.bfloat16)

        S = np.zeros((T, G), np.float32)
        for g, p in enumerate(gpos):
            p = int(p)
            if p // T == c:
                S[p % T, g] = 1.0
        d['ssel'] = np.ascontiguousarray(S.reshape(4, 128, G)).astype(
            ml_dtypes.bfloat16)

        SgT = np.zeros((G, T), np.float32)
        for p, g in last_slot.items():
            if p // T == c:
                SgT[g, p % T] = 1.0
        d['sgt'] = SgT

        # halo receive offsets into gath8 [8*1536, W]:
        # row = nbr*1536 + side*768 + p*6 + ft
        offs = np.full((128, 2, FT), OOB, np.int32)
        p_ar = np.arange(128)
        for combo in range(2):
            if combo == 0:              # left halo <- left neighbor's right edge
                if c == 0:
                    continue            # keys masked by inb
                nbr, side = c - 1, 1
            else:                       # right halo <- right neighbor's left edge
                if c == NCORE - 1:
                    continue
                nbr, side = c + 1, 0
            for ft in range(FT):
                offs[:, combo, ft] = nbr * 1536 + side * 768 + p_ar * 6 + ft
        # core 0 / 7 one-sided: keep junk reads in-bounds (row 0)
        offs[offs == OOB] = 0
        d['offs'] = offs
        per_core.append(d)

    return per_core, dict(ids=ids, am=am, gpos=gpos)


# ------------------------------------------------------------- the program

class Env:
    pass


def build_program(n_layers=N_LAYERS):
    nc = bacc.Bacc("TRN2", target_bir_lowering=False, debug=False,
                   enable_asserts=True, num_devices=NCORE)
    e = Env()
    e.nc = nc
    e.n_layers = n_layers

    def din(name, shape, dt=f32r):
        return nc.dram_tensor(name, list(shape), dt, kind="ExternalInput").ap()

    for n in ('wq', 'wk', 'wv', 'wqg', 'wkg', 'wvg', 'wo'):
        setattr(e, n, din(n, [NL, H, H], bf16))
    e.w1 = din('w1', [NL, H, FF], bf16)
    e.w2 = din('w2', [NL, FF, H], bf16)
    for n in ('bq', 'bk', 'bv', 'bqg', 'bkg', 'bvg', 'bo', 'b2',
              'ln1s', 'ln1b', 'ln2s', 'ln2b'):
        setattr(e, n, din(n, [NL, H], f32))
    e.b1 = din('b1', [NL, FF], f32)

    e.x0_ext = din('x0_ext', [FT, 128, T])
    e.x0b = din('x0b', [FT, 128, TEXT], bf16)
    e.xg0f_i = din('xg0f', [FT, 128, G], bf16)
    e.band01_i = din('band01', [NBLK, 6, 128, W], bf16)
    e.am01_i = din('am01k', [128, 4], f32)
    e.zrow_i = din('zrow2', [2, FT, T], bf16)
    e.ssel_i = din('ssel', [4, 128, G], bf16)
    e.sgt_i = din('sgt', [G, T])
    e.offs_i = din('offs', [128, 2, FT], i32)
    e.selh_i = din('selh', [2, 128], f32r)
    e.selh32_i = din('selh32', [2, 128], f32)
    e.onesr_i = din('onesr', [1, 128], f32r)
    e.cons_i = din('cons', [128, 1], f32r)
    e.cbf_i = din('cbf', [128, 128], bf16)
    e.eps_i = din('eps', [1, 1], f32)

    e.xout = nc.dram_tensor('xout', [FT, 128, T], f32r, kind="ExternalOutput").ap()

    with tile.TileContext(nc) as tc:
        e.tc = tc
        with contextlib.ExitStack() as stack:
            pers = stack.enter_context(tc.tile_pool(name="pers", bufs=1))
            dram = stack.enter_context(tc.tile_pool(name="dram", bufs=1, space="DRAM"))
            e.pers = pers
            e.dram = dram
            e.wpool = stack.enter_context(tc.tile_pool(name="wpool", bufs=3))
            e.wff = stack.enter_context(tc.tile_pool(name="wff", bufs=2))
            e.wnext = None

            e.x_ext = pers.tile([128, FT, T], f32r, tag="x_ext")
            e.xb = pers.tile([128, FT, TEXT], bf16, tag="xb")
            e.band_sb = pers.tile([128, NBLK, 6, W], bf16, tag="band")
            e.am01_sb = pers.tile([128, 4], f32, tag="am01")
            e.zrow_sb = pers.tile([2, FT, T], bf16, tag="zrow")
            e.ssel_sb = pers.tile([128, 4, G], bf16, tag="ssel")
            e.sgt_sb = pers.tile([64, T], f32r, tag="sgt")
            e.offs_sb = pers.tile([128, 2, FT], i32, tag="offs")
            e.selh_sb = pers.tile([2, 128], f32r, tag="selh")
            e.selh32_sb = pers.tile([2, 128], f32, tag="selh32")
            e.onesr_sb = pers.tile([1, 128], f32r, tag="onesr")
            e.ones_sb = pers.tile([128, 1], f32r, tag="ones")
            e.id_bf = pers.tile([128, 128], bf16, tag="idbf")
            e.eps_sb = pers.tile([1, 1], f32, tag="eps")

            nc.sync.dma_start(out=e.selh_sb[:, :], in_=e.selh_i[:, :])
            nc.sync.dma_start(out=e.selh32_sb[:, :], in_=e.selh32_i[:, :])
            nc.sync.dma_start(out=e.onesr_sb[:, :], in_=e.onesr_i[:, :])
            nc.sync.dma_start(out=e.ones_sb[:, :], in_=e.cons_i[:, :])
            nc.sync.dma_start(out=e.id_bf[:, :], in_=e.cbf_i[:, :])
            nc.sync.dma_start(out=e.eps_sb[:, :], in_=e.eps_i[:, :])

            for ft in range(FT):
                nc.sync.dma_start(out=e.x_ext[:, ft, :], in_=e.x0_ext[ft])
                nc.sync.dma_start(out=e.xb[:, ft, :], in_=e.x0b[ft])
            for b in range(NBLK):
                for j in range(6):
                    nc.sync.dma_start(out=e.band_sb[:, b, j, :],
                                     in_=e.band01_i[b, j])
            nc.sync.dma_start(out=e.am01_sb[:, :], in_=e.am01_i[:, :])
            nc.sync.dma_start(out=e.zrow_sb[:, :, :], in_=e.zrow_i[:, :, :])
            for kt in range(4):
                nc.sync.dma_start(out=e.ssel_sb[:, kt, :], in_=e.ssel_i[kt])
            nc.sync.dma_start(out=e.sgt_sb[:, :], in_=e.sgt_i[:, :])
            nc.sync.dma_start(out=e.offs_sb[:, :, :], in_=e.offs_i[:, :, :])

            e.edges_d = dram.tile([2, 128, FT, W], bf16, tag="edges")
            e.pc_d = dram.tile([H + 12, G], f32, tag="pc")
            e.xgc_d = dram.tile([H, G], f32, tag="xgc")

            for li in range(n_layers):
                with nc.named_scope(f"layer{li}"):
                    _layer(e, li)

            for ft in range(FT):
                nc.sync.dma_start(out=e.xout[ft], in_=e.x_ext[:, ft, :])

    nc.compile()
    return nc


def _bcast(ap, n):
    """Broadcast an AP along a new leading (partition) axis of size n."""
    return bass.AP(tensor=ap.tensor, offset=ap.offset, ap=[[0, n]] + list(ap.ap))


def _layer(e, li):
    nc, tc = e.nc, e.tc
    OWN = slice(W, W + T)

    with contextlib.ExitStack() as ctx:
        lay = ctx.enter_context(tc.tile_pool(name=f"lay{li}", bufs=1))
        wpool = e.wpool
        wff = e.wff
        sc = ctx.enter_context(tc.tile_pool(name=f"sc{li}", bufs=2))
        scl = ctx.enter_context(tc.tile_pool(name=f"scl{li}", bufs=1))
        pp = ctx.enter_context(tc.tile_pool(name=f"pp{li}", bufs=2))

        # ---- per-layer bias / ln param tiles
        def bias_tile(src, cols, tag):
            t = lay.tile([128, cols], f32, tag=tag)
            nc.sync.dma_start(out=t[:, :],
                             in_=src[li].rearrange("(f p) -> p f", p=128))
            return t

        bq_sb = bias_tile(e.bq, FT, "bq")
        bk_sb = bias_tile(e.bk, FT, "bk")
        bqg_sb = bias_tile(e.bqg, FT, "bqg")
        bkg_sb = bias_tile(e.bkg, FT, "bkg")
        bo_sb = bias_tile(e.bo, FT, "bo")
        b2_sb = bias_tile(e.b2, FT, "b2")
        b1_sb = bias_tile(e.b1, FF // 128, "b1")
        ln1s_sb = bias_tile(e.ln1s, FT, "ln1s")
        ln1b_sb = bias_tile(e.ln1b, FT, "ln1b")
        ln2s_sb = bias_tile(e.ln2s, FT, "ln2s")
        ln2b_sb = bias_tile(e.ln2b, FT, "ln2b")

        bv_exp = lay.tile([128, H], bf16, tag="bvexp")
        nc.gpsimd.dma_start(out=bv_exp[:, :], in_=_bcast(e.bv[li], 128))
        bvg_exp = lay.tile([128, H], bf16, tag="bvgexp")
        nc.gpsimd.dma_start(out=bvg_exp[:, :], in_=_bcast(e.bvg[li], 128))

        def load_w(src, lw=li):
            t = wpool.tile([128, FT, H], bf16, tag="wres")
            for kt in range(FT):
                nc.sync.dma_start(out=t[:, kt, :],
                                 in_=src[lw, kt * 128:(kt + 1) * 128, :])
            return t

        # ==================== P1: receive, projections, flash partials
        q_sb = lay.tile([128, FT, T], bf16, tag="q_z")
        k_ext = lay.tile([128, FT, TEXT], bf16, tag="k_ext")
        v_tok = lay.tile([128, 8, NH, DH + 2], bf16, tag="v_tok")
        vg2 = lay.tile([128, 4, NH, DH + 2], bf16, tag="vg2")
        vgs2 = lay.tile([64, NH, DH + 2], bf16, tag="vgs2")
        kg_own = lay.tile([128, FT, T], bf16, tag="kg_own")
        kgs_feat = lay.tile([128, FT, G], bf16, tag="kgs")
        qg_feat = lay.tile([128, FT, G], bf16, tag="qg")
        xg_feat = lay.tile([128, FT, G], bf16, tag="xg_feat")
        ctrb = lay.tile([128, FT, G], f32, tag="ctrb")
        cden = lay.tile([2, FT, G], f32, tag="cden")

        nc.vector.memset(v_tok[:, :, :, DH:DH + 2], 1.0)
        nc.vector.memset(vg2[:, :, :, DH:DH + 2], 1.0)
        nc.vector.memset(vgs2[:, :, DH:DH + 2], 1.0)

        with tc.tile_pool(name=f"P1_{li}", bufs=1, space="PSUM") as P1:
            # halo + global-token receive (DMA queues; wait on prev collectives)
            if li > 0:
                for combo, sl in enumerate([slice(0, W), slice(W + T, TEXT)]):
                    for ft in range(FT):
                        nc.gpsimd.indirect_dma_start(
                            out=e.xb[:, ft, sl],
                            out_offset=None,
                            in_=e.gath8_d[:, :],
                            in_offset=bass.IndirectOffsetOnAxis(
                                ap=e.offs_sb[:, combo, ft:ft + 1], axis=0),
                        )
                for ft in range(FT):
                    nc.gpsimd.dma_start(
                        out=xg_feat[:, ft, :],
                        in_=e.xgg_d[ft * 128:(ft + 1) * 128, :])
            else:
                for ft in range(FT):
                    nc.sync.dma_start(out=xg_feat[:, ft, :], in_=e.xg0f_i[ft])

            def proj_small(wres, bias, out):
                for ot in range(FT):
                    acc = P1.tile([128, G], f32, tag="accg")
                    for kt in range(FT):
                        nc.tensor.matmul(acc[:, :],
                                         wres[:, kt, ot * 128:(ot + 1) * 128],
                                         xg_feat[:, kt, :], start=(kt == 0),
                                         stop=(kt == FT - 1))
                    nc.vector.tensor_scalar_add(out=out[:, ot, :], in0=acc[:, :],
                                                scalar1=bias[:, ot:ot + 1])

            def proj_feat(wres, bias, out, src_cols, dst_cols, n):
                for ot in range(FT):
                    acc = P1.tile([128, 512], f32, tag="acc")
                    for kt in range(FT):
                        nc.tensor.matmul(acc[:, :n],
                                         wres[:, kt, ot * 128:(ot + 1) * 128],
                                         e.xb[:, kt, src_cols],
                                         start=(kt == 0), stop=(kt == FT - 1))
                    nc.vector.tensor_scalar_add(out=out[:, ot, dst_cols],
                                                in0=acc[:, :n],
                                                scalar1=bias[:, ot:ot + 1])

            def proj_tok(wres, bias_exp, out, tchunks, col0):
                # out: [128, nchunk, NH, DH+1]; writes the DH feature columns
                for tc_ in tchunks:
                    for half in range(2):
                        hs = slice(half * 384, (half + 1) * 384)
                        acc = P1.tile([128, 384], f32, tag="acc")
                        cs = slice(col0 + tc_ * 128, col0 + (tc_ + 1) * 128)
                        for kt in range(FT):
                            nc.tensor.matmul(acc[:, :], e.xb[:, kt, cs],
                                             wres[:, kt, hs], start=(kt == 0),
                                             stop=(kt == FT - 1))
                        nc.vector.tensor_add(
                            out=out[:, tc_, half * 6:(half + 1) * 6, 0:DH],
                            in0=acc[:, :], in1=bias_exp[:, hs])

            # 1. own-x projections (no external deps)
            if e.wnext is not None:
                wres, e.wnext = e.wnext, None
            else:
                wres = load_w(e.wq)
            proj_feat(wres, bq_sb, q_sb, OWN, slice(0, T), T)

            wres = load_w(e.wkg)
            proj_feat(wres, bkg_sb, kg_own, OWN, slice(0, T), T)
            proj_small(wres, bkg_sb, kgs_feat)

            wres = load_w(e.wvg)
            proj_tok(wres, bvg_exp, vg2, range(4), W)
            for half in range(2):
                hs = slice(half * 384, (half + 1) * 384)
                acc = P1.tile([64, 384], f32, tag="accW")
                for kt in range(FT):
                    nc.tensor.matmul(acc[:, :], xg_feat[:, kt, :], wres[:, kt, hs],
                                     start=(kt == 0), stop=(kt == FT - 1))
                nc.vector.tensor_add(
                    out=vgs2[:, half * 6:(half + 1) * 6, 0:DH],
                    in0=acc[:, :], in1=bvg_exp[0:64, hs])

            wres = load_w(e.wqg)
            proj_small(wres, bqg_sb, qg_feat)

            # 2. flash partials for global query rows (key-major, max-free)
            pfs = {}

            def flash_scores(h):
                hp, ht = (h % 2) * 64, h // 2
                fT = P1.tile([128, 4, G], f32, tag=f"fT{h % 2}")
                for j in range(4):
                    nc.tensor.matmul(fT[:, j, :],
                                     kg_own[hp:hp + 64, ht, j * 128:(j + 1) * 128],
                                     qg_feat[hp:hp + 64, ht, :],
                                     start=True, stop=True)
                return fT

            def flash_soft(h, fT):
                pf = sc.tile([128, 4, G], bf16, tag="pf")
                nc.scalar.activation(out=pf[:, :, :], in_=fT[:, :, :],
                                     func=Act.Exp, bias=0.0, scale=SCALE)
                for j in range(4):
                    nc.vector.tensor_scalar_mul(out=pf[:, j, :], in0=pf[:, j, :],
                                                scalar1=e.am01_sb[:, j:j + 1])
                pfs[h] = pf

            def flash_av(h):
                hp, ht = (h % 2) * 64, h // 2
                ovf = P1.tile([DH + 2, G], f32, tag=f"ovf{h % 2}")
                for j in range(4):
                    nc.tensor.matmul(ovf[:, :], vg2[:, j, h, :], pfs[h][:, j, :],
                                     start=(j == 0), stop=(j == 3),
                                     skip_group_check=True)
                nc.vector.tensor_copy(out=ctrb[hp:hp + 64, ht, :],
                                      in_=ovf[0:64, :])
                # odd head -> partitions 0:2 (slot 0 scratch), even -> 0:1
                if h % 2:
                    nc.scalar.copy(out=cden[0:2, ht, :], in_=ovf[DH:DH + 2, :])
                else:
                    nc.scalar.copy(out=cden[0:1, ht, :], in_=ovf[DH:DH + 1, :])

            horder = [1, 0, 3, 2, 5, 4, 7, 6, 9, 8, 11, 10]
            fT = flash_scores(horder[0])
            for hi, h in enumerate(horder):
                nf = flash_scores(horder[hi + 1]) if hi + 1 < NH else None
                flash_soft(h, fT)
                flash_av(h)
                fT = nf

            for ft in range(FT):
                nc.sync.dma_start(out=e.pc_d[ft * 128:(ft + 1) * 128, :],
                                 in_=ctrb[:, ft, :])
            nc.sync.dma_start(out=e.pc_d[H:H + 12, :], in_=cden[:, :, :])
            e.pg_d = e.dram.tile([H + 12, G], f32, tag=f"pg{li}",
                                 addr_space="Shared")
            nc.gpsimd.collective_compute(
                "AllReduce", Alu.add, replica_groups=AG8,
                ins=[e.pc_d.opt()], outs=[e.pg_d.opt()])

            # 3. k / v over own + halo tokens (halo newly received)
            wres = load_w(e.wv)
            proj_tok(wres, bv_exp, v_tok, [2, 3, 4, 5, 1, 6, 0, 7], 0)

            wres = load_w(e.wk)
            proj_feat(wres, bk_sb, k_ext, OWN, slice(W, W + T), T)
            proj_feat(wres, bk_sb, k_ext, slice(0, W), slice(0, W), W)
            proj_feat(wres, bk_sb, k_ext, slice(W + T, TEXT), slice(W + T, TEXT), W)

        # ==================== P2: local attention (key-major, pipelined)
        attn_raw = lay.tile([128, FT, T], bf16, tag="x_mid")
        out_attn = lay.tile([128, FT, T], bf16, tag="attnb")
        rz_all = lay.tile([2, FT, T], f32, tag="rz_all")
        rz32 = lay.tile([2, FT, T], f32, tag="kg_own")
        items = [(b, h) for b in range(NBLK)
         for h in (1, 0, 3, 2, 5, 4, 7, 6, 9, 8, 11, 10)]
        with tc.tile_pool(name=f"P2a_{li}", bufs=2, space="PSUM") as P2a, \
             tc.tile_pool(name=f"P2b_{li}", bufs=1, space="PSUM") as P2b:
            sTs, sgs, pTs, pTgs = {}, {}, {}, {}

            def p2_scores(i):
                b, h = items[i]
                hp, ht = (h % 2) * 64, h // 2
                qs = slice(b * W, (b + 1) * W)
                sT = P2a.tile([128, 6, W], f32, tag="sT")
                for j in range(6):
                    nc.tensor.matmul(
                        sT[:, j, :],
                        k_ext[hp:hp + 64, ht, b * W + j * 128:b * W + j * 128 + 128],
                        q_sb[hp:hp + 64, ht, qs], start=True, stop=True)
                sg = P2b.tile([64, W], f32, tag="sg")
                nc.tensor.matmul(sg[:, :], kgs_feat[hp:hp + 64, ht, :],
                                 q_sb[hp:hp + 64, ht, qs], start=True, stop=True)
                sTs[i], sgs[i] = sT, sg

            def p2_soft(i):
                b, h = items[i]
                pT = sc.tile([128, 6, W], bf16, tag="pT")
                nc.scalar.activation(out=pT[:, :, :], in_=sTs[i][:, :, :],
                                     func=Act.Exp, bias=0.0, scale=SCALE)
                nc.vector.tensor_tensor(out=pT[:, :, :], in0=pT[:, :, :],
                                        in1=e.band_sb[:, b, :, :], op=Alu.mult)
                pTg = sc.tile([64, W], bf16, tag="pTg")
                nc.scalar.activation(out=pTg[:, :], in_=sgs[i][:, :],
                                     func=Act.Exp, bias=0.0, scale=SCALE)
                pTs[i], pTgs[i] = pT, pTg

            def p2_av(i):
                b, h = items[i]
                hp, ht = (h % 2) * 64, h // 2
                qs = slice(b * W, (b + 1) * W)
                ov = P2b.tile([DH + 2, W], f32, tag="ov")
                for j in range(6):
                    nc.tensor.matmul(ov[:, :], v_tok[:, 2 * b + j, h, :],
                                     pTs[i][:, j, :], start=(j == 0), stop=False,
                                     skip_group_check=True)
                nc.tensor.matmul(ov[:, :], vgs2[:, h, :], pTgs[i][:, :],
                                 start=False, stop=True, skip_group_check=True)
                nc.vector.tensor_copy(out=attn_raw[hp:hp + 64, ht, qs],
                                      in_=ov[0:DH, :])
                if h % 2:
                    nc.scalar.copy(out=rz_all[0:2, ht, qs],
                                   in_=ov[DH:DH + 2, :])
                else:
                    nc.scalar.copy(out=rz_all[0:1, ht, qs],
                                   in_=ov[DH:DH + 1, :])

            p2_scores(0)
            for i in range(len(items)):
                if i + 1 < len(items):
                    p2_scores(i + 1)
                p2_soft(i)
                p2_av(i)

        # normalize: out = raw * bcast(zrow / denom)
        with tc.tile_pool(name=f"P2n_{li}", bufs=2, space="PSUM") as P2n:
            nc.vector.reciprocal_approx_fast(out=rz32[:, :, :],
                                             in_=rz_all[:, :, :])
            nc.vector.tensor_tensor(out=rz_all[:, :, :], in0=rz32[:, :, :],
                                    in1=e.zrow_sb[:, :, :], op=Alu.mult)
            for ft in range(FT):
                dx = P2n.tile([128, T], f32, tag="dexp")
                nc.tensor.matmul(dx[:, :], e.selh32_sb[:, :], rz_all[:, ft, :],
                                 start=True, stop=True)
                nc.vector.tensor_tensor(out=out_attn[:, ft, :],
                                        in0=attn_raw[:, ft, :], in1=dx[:, :],
                                        op=Alu.mult)

        # ==================== P3: combine global rows, Wo, LN1
        z_sb = lay.tile([128, FT, T], f32r, tag="q_z")
        x_mid = lay.tile([128, FT, T], f32r, tag="x_mid")
        xb_mid = lay.tile([128, FT, T], bf16, tag="xb_mid")
        with tc.tile_pool(name=f"P3_{li}", bufs=1, space="PSUM") as P3:
            og_raw = lay.tile([128, FT, G], f32, tag="og_raw")
            for ft in range(FT):
                nc.sync.dma_start(out=og_raw[:, ft, :],
                                 in_=e.pg_d[ft * 128:(ft + 1) * 128, :])
            grz = scl.tile([2, FT, G], f32, tag="grz")
            nc.sync.dma_start(
                out=grz[:, :, :],
                in_=e.pg_d[H:H + 12, :].rearrange("(two f) g -> two f g", two=2))
            nc.vector.reciprocal_approx_fast(out=grz[:, :, :], in_=grz[:, :, :])
            og_feat = lay.tile([128, FT, G], bf16, tag="og_feat")
            for ft in range(FT):
                dg = P3.tile([128, G], f32, tag="dg")
                nc.tensor.matmul(dg[:, :], e.selh32_sb[:, :], grz[:, ft, :],
                                 start=True, stop=True)
                nc.vector.tensor_tensor(out=og_feat[:, ft, :],
                                        in0=og_raw[:, ft, :], in1=dg[:, :],
                                        op=Alu.mult)

            wres = load_w(e.wo)
            og_w = lay.tile([64, H], f32r, tag="og_w")
            for half in range(2):
                hs = slice(half * 384, (half + 1) * 384)
                acc = P3.tile([64, 384], f32, tag="accW")
                for kt in range(FT):
                    nc.tensor.matmul(acc[:, :], og_feat[:, kt, :], wres[:, kt, hs],
                                     start=(kt == 0), stop=(kt == FT - 1))
                nc.scalar.copy(out=og_w[:, hs], in_=acc[:, :])

            for mt in range(FT):
                acc = P3.tile([128, T], f32, tag="acc")
                for kt in range(FT):
                    nc.tensor.matmul(acc[:, :],
                                     wres[:, kt, mt * 128:(mt + 1) * 128],
                                     out_attn[:, kt, :], start=(kt == 0),
                                     stop=False, skip_group_check=True)
                nc.tensor.matmul(acc[:, :], og_w[:, mt * 128:(mt + 1) * 128],
                                 e.sgt_sb[:, :], start=False, stop=True,
                                 skip_group_check=True)
                nc.vector.tensor_scalar_add(out=z_sb[:, mt, :], in0=acc[:, :],
                                            scalar1=bo_sb[:, mt:mt + 1])
                nc.vector.tensor_add(out=z_sb[:, mt, :], in0=z_sb[:, mt, :],
                                     in1=e.x_ext[:, mt, :])

            _layernorm(e, z_sb, x_mid, None, ln1s_sb, ln1b_sb, scl, lay, P3,
                       bf_out=xb_mid)

        # ==================== P4: FFN
        z2 = lay.tile([128, FT, T], f32r, tag="q_z")
        with tc.tile_pool(name=f"P4_{li}", bufs=1, space="PSUM") as P4:
            y2 = P4.tile([128, FT, T], f32, tag="y2")
            for ot in range(FF // 128):
                w1s = wff.tile([128, FT, 128], bf16, tag="w1s")
                nc.sync.dma_start(
                    out=w1s[:, :, :],
                    in_=e.w1[li, :, ot * 128:(ot + 1) * 128].rearrange(
                        "(k p) o -> p k o", p=128))
                hps = P4.tile([128, T], f32, tag=f"h{ot % 2}")
                for kt in range(FT):
                    nc.tensor.matmul(hps[:, :], w1s[:, kt, :], xb_mid[:, kt, :],
                                     start=(kt == 0), stop=(kt == FT - 1))
                h_sb = pp.tile([128, T], bf16, tag="h_sb")
                nc.scalar.activation(out=h_sb[:, :], in_=hps[:, :], func=Act.Gelu,
                                     bias=b1_sb[:, ot:ot + 1], scale=1.0)
                w2s = wff.tile([128, H], bf16, tag="w2s")
                nc.scalar.dma_start(out=w2s[:, :],
                                 in_=e.w2[li, ot * 128:(ot + 1) * 128, :])
                for mt in range(FT):
                    nc.tensor.matmul(y2[:, mt, :],
                                     w2s[:, mt * 128:(mt + 1) * 128],
                                     h_sb[:, :], start=(ot == 0),
                                     stop=(ot == FF // 128 - 1),
                                     skip_group_check=True)
            for mt in range(FT):
                nc.vector.tensor_scalar_add(out=z2[:, mt, :], in0=y2[:, mt, :],
                                            scalar1=b2_sb[:, mt:mt + 1])
                nc.vector.tensor_add(out=z2[:, mt, :], in0=z2[:, mt, :],
                                     in1=x_mid[:, mt, :])

        if li + 1 < e.n_layers:
            e.wnext = load_w(e.wq, li + 1)

        # ==================== P5: LN2 (writes x/xb own), edges + xg collectives
        with tc.tile_pool(name=f"P5_{li}", bufs=1, space="PSUM") as P5:
            _layernorm(e, z2, None, slice(0, T), ln2s_sb, ln2b_sb, scl, lay,
                       P5, bf_out=e.xb, bf_cols=OWN)

            if li < e.n_layers - 1:
                for side, sl in ((0, slice(W, 2 * W)), (1, slice(T, W + T))):
                    nc.gpsimd.dma_start(out=e.edges_d[side], in_=e.xb[:, :, sl])
                e.gath8_d = e.dram.tile([NCORE * 1536, W], bf16,
                                        tag=f"g8_{li}", addr_space="Shared")
                nc.gpsimd.collective_compute(
                    "AllGather", Alu.bypass, replica_groups=AG8,
                    ins=[e.edges_d.opt()], outs=[e.gath8_d.opt()])

                x_tok = lay.tile([128, 4, H], bf16, tag="x_tok")
                for j in range(4):
                    for ft in range(FT):
                        tp = P5.tile([128, 128], bf16, tag="tp")
                        nc.tensor.transpose(
                            out=tp[:, :],
                            in_=e.xb[:, ft, W + j * 128:W + (j + 1) * 128],
                            identity=e.id_bf)
                        nc.scalar.copy(out=x_tok[:, j, ft * 128:(ft + 1) * 128],
                                       in_=tp[:, :])
                xgc_sb = lay.tile([128, FT, G], f32, tag="xgc_sb")
                for ft in range(FT):
                    xgp = P5.tile([128, G], f32, tag="xgp")
                    for j in range(4):
                        nc.tensor.matmul(xgp[:, :],
                                         x_tok[:, j, ft * 128:(ft + 1) * 128],
                                         e.ssel_sb[:, j, :], start=(j == 0),
                                         stop=(j == 3))
                    nc.scalar.copy(out=xgc_sb[:, ft, :], in_=xgp[:, :])
                for ft in range(FT):
                    nc.sync.dma_start(out=e.xgc_d[ft * 128:(ft + 1) * 128, :],
                                     in_=xgc_sb[:, ft, :])
                e.xgg_d = e.dram.tile([H, G], f32, tag=f"xgg{li}",
                                      addr_space="Shared")
                nc.gpsimd.collective_compute(
                    "AllReduce", Alu.add, replica_groups=AG8,
                    ins=[e.xgc_d.opt()], outs=[e.xgg_d.opt()])


def _layernorm(e, z_sb, out, own_slice, s_t, b_t, scl, lay, P, bf_out=None,
               bf_cols=slice(None)):
    """LN over features (feature-major). out=None -> write x_ext own."""
    nc = e.nc
    s1 = P.tile([1, T], f32, tag="stat1")
    for kt in range(FT):
        nc.tensor.matmul(s1[:, :], e.ones_sb, z_sb[:, kt, :],
                         start=(kt == 0), stop=(kt == FT - 1))
    s2 = P.tile([1, T], f32, tag="stat2")
    for kt in range(FT):
        zsq = lay.tile([128, T], f32r, tag="zsq")
        nc.scalar.activation(out=zsq[:, :], in_=z_sb[:, kt, :],
                             func=Act.Square, bias=0.0, scale=1.0)
        nc.tensor.matmul(s2[:, :], e.ones_sb, zsq[:, :],
                         start=(kt == 0), stop=(kt == FT - 1))
    A = scl.tile([1, T], f32r, tag="lnA")   # mean
    B = scl.tile([1, T], f32r, tag="lnB")   # msq -> var -> sd -> rstd
    C = scl.tile([1, T], f32r, tag="lnC")   # mean*rstd
    nc.vector.tensor_scalar_mul(out=A[:, :], in0=s1[:, :], scalar1=1.0 / H)
    nc.vector.tensor_scalar_mul(out=B[:, :], in0=s2[:, :], scalar1=1.0 / H)
    nc.vector.tensor_tensor(out=C[:, :], in0=A[:, :], in1=A[:, :], op=Alu.mult)
    nc.vector.tensor_tensor(out=B[:, :], in0=B[:, :], in1=C[:, :], op=Alu.subtract)
    nc.scalar.activation(out=B[:, :], in_=B[:, :], func=Act.Sqrt,
                         bias=e.eps_sb[:, :], scale=1.0)
    with nc.allow_low_precision(reason="f32r == f32 bits"):
        nc.vector.reciprocal(out=B[:, :], in_=B[:, :])
    nc.vector.tensor_tensor(out=C[:, :], in0=A[:, :], in1=B[:, :], op=Alu.mult)
    rstd_b = P.tile([128, T], f32, tag="lnbc1")
    nc.tensor.matmul(rstd_b[:, :], e.onesr_sb[:, :], B[:, :],
                     start=True, stop=True)
    mrs_b = P.tile([128, T], f32, tag="lnbc2")
    nc.tensor.matmul(mrs_b[:, :], e.onesr_sb[:, :], C[:, :],
                     start=True, stop=True)
    for mt in range(FT):
        dst = e.x_ext[:, mt, own_slice] if out is None else out[:, mt, :]
        nc.vector.tensor_tensor(out=dst, in0=z_sb[:, mt, :], in1=rstd_b[:, :],
                                op=Alu.mult)
        nc.vector.tensor_tensor(out=dst, in0=dst, in1=mrs_b[:, :],
                                op=Alu.subtract)
        nc.vector.tensor_scalar(out=dst, in0=dst, scalar1=s_t[:, mt:mt + 1],
                                scalar2=b_t[:, mt:mt + 1], op0=Alu.mult,
                                op1=Alu.add)
        if bf_out is not None:
            nc.scalar.copy(out=bf_out[:, mt, bf_cols], in_=dst)


# ---------------------------------------------------------------- driver

_CACHE = {}


def _get_program():
    if N_LAYERS not in _CACHE:
        _CACHE[N_LAYERS] = build_program(N_LAYERS)
    return _CACHE[N_LAYERS]


def kernel(**inputs):
    import ml_dtypes
    per_core, host = host_prep(inputs)
    nc = _get_program()

    wsrc = {'wq': 'Wq', 'wk': 'Wk', 'wv': 'Wv', 'wqg': 'Wqg', 'wkg': 'Wkg',
            'wvg': 'Wvg', 'wo': 'Wo', 'w1': 'W1', 'w2': 'W2'}
    bsrc = {'bq': 'bq', 'bk': 'bk', 'bv': 'bv', 'bqg': 'bqg', 'bkg': 'bkg',
            'bvg': 'bvg', 'bo': 'bo', 'b1': 'b1', 'b2': 'b2',
            'ln1s': 'ln1_s', 'ln1b': 'ln1_b', 'ln2s': 'ln2_s', 'ln2b': 'ln2_b'}
    shared = {k: np.ascontiguousarray(
                  np.asarray(inputs[v], np.float32).astype(ml_dtypes.bfloat16))
              for k, v in wsrc.items()}
    shared.update({k: np.ascontiguousarray(np.asarray(inputs[v], np.float32))
                   for k, v in bsrc.items()})

    selh = np.zeros((2, 128), np.float32)
    selh[0, 0:64] = 1.0
    selh[1, 64:128] = 1.0
    shared['selh'] = selh
    shared['selh32'] = selh
    shared['onesr'] = np.ones((1, 128), np.float32)
    shared['cons'] = np.ones((128, 1), np.float32)
    shared['cbf'] = np.eye(128).astype(ml_dtypes.bfloat16)
    shared['eps'] = np.full((1, 1), 1e-5, np.float32)

    in_maps = []
    for c in range(NCORE):
        m = dict(shared)
        d = per_core[c]
        for k in ('x0_ext', 'x0b', 'xg0f', 'band01', 'am01k', 'zrow2', 'ssel',
                  'sgt', 'offs'):
            m[k] = d[k]
        in_maps.append(m)

    trace = bool(int(os.environ.get("KERNEL_TRACE", "0")))
    res = run_bass_kernel_spmd(nc, in_maps, core_ids=list(range(NCORE)),
                               trace=trace)
    kernel.last_result = res

    x = np.zeros((L, H), np.float32)
    for c in range(NCORE):
        xo = res.results[c]['xout']
        x[c * T:(c + 1) * T] = xo.transpose(2, 0, 1).reshape(T, H)
    kernel.last_x = x

    ids = host['ids'][0]
    cand_mask = ids == int(np.asarray(inputs['cand_token_id']))
    order = np.argsort(np.where(cand_mask, 0, 1).astype(np.int32), kind='stable')
    positions = order[:CMAX]
    valid = cand_mask[positions]
    g = x[positions]
    hh = _np_gelu(g @ np.asarray(inputs['Wh1'], np.float32)
                  + np.asarray(inputs['bh1'], np.float32))
    logits = (hh @ np.asarray(inputs['Wh2'], np.float32)
              + np.asarray(inputs['bh2'], np.float32))[:, 0]
    return logits[None].astype(np.float32), valid[None]


if __name__ == '__main__':
    print("building program ...")
    build_program(N_LAYERS)
    print("build OK")
